# revision 12
# baseline (speedup 1.0000x reference)
"""EGNN continuous ODE on 8 Trainium2 NeuronCores (Bass/Tile).

Nodes sharded 8-way (per-core class-ordered with padding), edges sharded by
row-core into an 8-slot virtual-node grid. Per Euler step one SPMD launch:
B-side rows fetched with 128-row indirect DMA gathers + PE transposes,
edge MLP feature-major on PE, slot aggregation via PSUM-accumulating matmuls,
virtual->real combine, node MLP, and next-step table build. Host glues the
4 launches (global B-table assembly, virtual expansion, permutations).
"""
import sys
sys.path.insert(0, '/opt/trn_rl_repo')
import numpy as np
from concourse import bass, tile, mybir, bass_utils, bacc

H = 64
F = 128
N = 50000
NC = 8
NLOC = 6250
NPAD = 6656          # padded real nodes per core (13*512)
SLOTS = 8
VW = 512
SLOPE = 0.3
NWIN = 32
NVV = NWIN * VW      # 16384 virtual nodes padded
ES = NVV * SLOTS     # 131072 slot stream
NCALL = ES // 128    # 1024 gather calls / step
FP = mybir.dt.float32
DT = float(2.0 / 3.0)
TRACE = False
LAST_EXEC_NS = None


def _vdeg_of(row_l):
    deg = np.bincount(row_l, minlength=NPAD)
    return deg, np.maximum((deg + SLOTS - 1) // SLOTS, 1)


def _prep_core(row_l, col_g, eids, edge_attr, We, classes_u):
    deg, vdeg = _vdeg_of(row_l)
    # build class-ordered real-node permutation under the UNIFORM layout
    order = np.full(NPAD, -1, np.int64)     # class-pos -> orig local id
    by_k = {}
    for i in np.argsort(vdeg[:NLOC], kind="stable"):
        by_k.setdefault(int(vdeg[i]), []).append(int(i))
    pad_pool = list(range(NLOC, NPAD))      # dummy real nodes (deg 0)
    v2r = np.zeros(NVV, np.int64)           # virtual -> class-pos of real node
    for (k, rstart, vstart, nk) in classes_u:
        mine = by_k.get(k, [])
        assert len(mine) <= nk, (k, len(mine), nk)
        for i, orig in enumerate(mine):
            order[rstart + i] = orig
        for i in range(len(mine), nk):
            order[rstart + i] = pad_pool.pop()
        for i in range(nk * k):
            v2r[vstart + i] = rstart + i // k
    # leftover class positions (beyond classes_u coverage) get remaining pads
    for i in range(NPAD):
        if order[i] < 0:
            order[i] = pad_pool.pop()
    inv_order = np.argsort(order)
    # slot assignment for real edges
    first_v = np.zeros(NPAD, np.int64)      # orig local -> first virtual id
    cls_pos_of = inv_order                  # orig local -> class pos
    # first virtual of each class position
    firstv_of_cpos = np.zeros(NPAD, np.int64)
    for (k, rstart, vstart, nk) in classes_u:
        firstv_of_cpos[rstart:rstart + nk] = vstart + np.arange(nk) * k
    first_v = firstv_of_cpos[cls_pos_of]
    srt = np.argsort(row_l, kind="stable")
    rl_s, cg_s, eid_s = row_l[srt], col_g[srt], eids[srt]
    first_e = np.zeros(NPAD + 1, np.int64)
    np.cumsum(deg, out=first_e[1:])
    within = np.arange(rl_s.size) - first_e[rl_s]
    virt = first_v[rl_s] + within // SLOTS
    slot = within % SLOTS
    pos = (virt // VW) * (SLOTS * VW) + slot * VW + (virt % VW)
    gidx = np.zeros(ES, np.int64)
    mask = np.zeros(ES, np.float32)
    ep = np.zeros((ES, H), np.float32)
    gidx[pos] = cg_s
    mask[pos] = 1.0
    ep[pos] = edge_attr[eid_s] @ We
    cnt = np.maximum(deg, 1).astype(np.float32)[order]   # class order
    return dict(order=order, inv_order=inv_order, v2r=v2r,
                gidx=gidx, mask=mask, eproj=ep, cnt=cnt)


def _build_wab(t, eW1, eb1):
    Wa = np.zeros((71, 68), np.float32)
    Wb = np.zeros((71, 68), np.float32)
    Wr = eW1[130]
    Wa[0, :64] = t * eW1[0] + eb1
    Wa[1:65, :64] = eW1[1:65]
    Wa[65:68, 64:67] = np.eye(3)
    Wa[68:71, :64] = Wr
    Wb[0, :64] = t * eW1[65]
    Wb[1:65, :64] = eW1[66:130]
    Wb[65:68, 64:67] = np.eye(3)
    Wb[68:71, :64] = Wr
    return Wa, Wb


def _lrelu(nc, out_ap, in_ap, pool, shape):
    tmp = pool.tile(list(shape), FP, tag="lrt")
    nc.scalar.mul(out=tmp[:], in_=in_ap, mul=SLOPE)
    nc.vector.tensor_tensor(out=out_ap, in0=in_ap, in1=tmp[:], op=mybir.AluOpType.max)


def _build_ext(nc, pool, h_ap, p_ap, ones_ap):
    ext = pool.tile([71, NPAD], FP, tag="ext")
    nc.vector.tensor_copy(out=ext[0:1, :], in_=ones_ap)
    nc.vector.tensor_copy(out=ext[1:65, :], in_=h_ap)
    nc.vector.tensor_copy(out=ext[65:68, :], in_=p_ap)
    nc.vector.tensor_tensor(out=ext[68:71, :], in0=p_ap, in1=p_ap, op=mybir.AluOpType.mult)
    return ext


def _table_mms(nc, sbuf, psum, ext_t, wab_t, outA, outB):
    for b in range(NPAD // 128):
        pa = psum.tile([128, 68], FP, tag="pe")
        lhsT = ext_t[:, b * 128:(b + 1) * 128]
        nc.tensor.matmul(out=pa[:], lhsT=lhsT, rhs=wab_t[:, 0:68], start=True, stop=True)
        sa = sbuf.tile([128, 68], FP, tag="wE")
        nc.vector.tensor_copy(out=sa[:], in_=pa[:])
        nc.sync.dma_start(out=outA[b * 128:(b + 1) * 128, :], in_=sa[:])
        pb = psum.tile([128, 68], FP, tag="pe")
        nc.tensor.matmul(out=pb[:], lhsT=lhsT, rhs=wab_t[:, 68:136], start=True, stop=True)
        sb2 = sbuf.tile([128, 68], FP, tag="wV")
        nc.vector.tensor_copy(out=sb2[:], in_=pb[:])
        nc.sync.dma_start(out=outB[b * 128:(b + 1) * 128, :], in_=sb2[:])


def build_p0():
    nc = bacc.Bacc("TRN2", target_bir_lowering=False, debug=False, num_devices=NC)
    x_in = nc.dram_tensor("x_fm", [128, NPAD], FP, kind="ExternalInput")
    pos_in = nc.dram_tensor("pos_fm", [3, NPAD], FP, kind="ExternalInput")
    embW_in = nc.dram_tensor("embW", [128, 64], FP, kind="ExternalInput")
    embB_in = nc.dram_tensor("embB", [64, 1], FP, kind="ExternalInput")
    wab0_in = nc.dram_tensor("wab0", [1, 136], FP, kind="ExternalInput")
    wabh_in = nc.dram_tensor("wabh", [64, 136], FP, kind="ExternalInput")
    wabp_in = nc.dram_tensor("wabp", [3, 136], FP, kind="ExternalInput")
    wabq_in = nc.dram_tensor("wabq", [3, 136], FP, kind="ExternalInput")
    ones_in = nc.dram_tensor("ones", [1, NPAD], FP, kind="ExternalInput")
    h0_out = nc.dram_tensor("h0", [64, NPAD], FP, kind="ExternalOutput")
    tabA_out = nc.dram_tensor("tabA", [NPAD, 68], FP, kind="ExternalOutput")
    tabB_out = nc.dram_tensor("tabB", [NPAD, 68], FP, kind="ExternalOutput")
    with tile.TileContext(nc) as tc:
        with tc.tile_pool(name="sb", bufs=1) as sbuf, \
             tc.tile_pool(name="ps", bufs=2, space="PSUM") as psum, \
             tc.tile_pool(name="sb2", bufs=2) as sbuf2:
            x_t = sbuf.tile([128, NPAD], FP)
            pos_t = sbuf.tile([3, NPAD], FP)
            embW_t = sbuf.tile([128, 64], FP)
            embB_t = sbuf.tile([64, 1], FP)
            wab0_t = sbuf.tile([1, 136], FP)
            wabh_t = sbuf.tile([64, 136], FP)
            wabp_t = sbuf.tile([3, 136], FP)
            wabq_t = sbuf.tile([3, 136], FP)
            ones_t = sbuf.tile([1, NPAD], FP)
            h0_t = sbuf.tile([64, NPAD], FP)
            for ap, src in [(x_t, x_in), (pos_t, pos_in), (embW_t, embW_in),
                            (embB_t, embB_in), (wab0_t, wab0_in), (wabh_t, wabh_in),
                            (wabp_t, wabp_in), (wabq_t, wabq_in), (ones_t, ones_in)]:
                nc.sync.dma_start(out=ap[:], in_=src[:])
            for b in range(NPAD // 512):
                pe = psum.tile([64, 512], FP, tag="pe")
                nc.tensor.matmul(out=pe[:], lhsT=embW_t[:],
                                 rhs=x_t[:, b * 512:(b + 1) * 512], start=True, stop=True)
                nc.vector.tensor_tensor(out=h0_t[:, b * 512:(b + 1) * 512],
                                        in0=pe[:], in1=embB_t[:].to_broadcast([64, 512]),
                                        op=mybir.AluOpType.add)
            for b in range(NPAD // 128):
                cs = slice(b * 128, (b + 1) * 128)
                psq = sbuf2.tile([3, 128], FP, tag="psq")
                nc.vector.tensor_tensor(out=psq[:], in0=pos_t[:, cs], in1=pos_t[:, cs], op=mybir.AluOpType.mult)
                for (rhs_sl, outT, tag) in [(slice(0, 68), tabA_out, "tsA"), (slice(68, 136), tabB_out, "tsB")]:
                    pa = psum.tile([128, 68], FP, tag="pe2")
                    nc.tensor.matmul(out=pa[:], lhsT=ones_t[:, cs], rhs=wab0_t[:, rhs_sl], start=True, stop=False)
                    nc.tensor.matmul(out=pa[:], lhsT=h0_t[:, cs], rhs=wabh_t[:, rhs_sl], start=False, stop=False)
                    nc.tensor.matmul(out=pa[:], lhsT=pos_t[:, cs], rhs=wabp_t[:, rhs_sl], start=False, stop=False)
                    nc.tensor.matmul(out=pa[:], lhsT=psq[:], rhs=wabq_t[:, rhs_sl], start=False, stop=True)
                    sa = sbuf2.tile([128, 68], FP, tag=tag)
                    nc.vector.tensor_copy(out=sa[:], in_=pa[:])
                    nc.sync.dma_start(out=outT[cs, :], in_=sa[:])
            nc.sync.dma_start(out=h0_out[:], in_=h0_t[:])
    nc.compile()
    return nc


def build_ps(classes_u):
    nc = bacc.Bacc("TRN2", target_bir_lowering=False, debug=False, num_devices=NC)
    h_in = nc.dram_tensor("h", [64, NPAD], FP, kind="ExternalInput")
    p_in = nc.dram_tensor("p", [3, NPAD], FP, kind="ExternalInput")
    av_in = nc.dram_tensor("avirt", [128, (NVV // 128) * 68], FP, kind="ExternalInput")
    bt_in = nc.dram_tensor("btab", [NC * NPAD, 68], FP, kind="ExternalInput")
    gi_in = nc.dram_tensor("gidx", [128, NCALL], mybir.dt.int32, kind="ExternalInput")
    maskE_in = nc.dram_tensor("maskE", [16, ES], FP, kind="ExternalInput")
    maskT_in = nc.dram_tensor("maskT", [3, ES], FP, kind="ExternalInput")
    ep_in = nc.dram_tensor("eproj", [64, ES], FP, kind="ExternalInput")
    trow_in = nc.dram_tensor("trow", [1, 512], FP, kind="ExternalInput")
    wab0_in = nc.dram_tensor("wab0", [1, 136], FP, kind="ExternalInput")
    wabh_in = nc.dram_tensor("wabh", [64, 136], FP, kind="ExternalInput")
    wabp_in = nc.dram_tensor("wabp", [3, 136], FP, kind="ExternalInput")
    wabq_in = nc.dram_tensor("wabq", [3, 136], FP, kind="ExternalInput")
    ew2_in = nc.dram_tensor("ew2", [64, 32], FP, kind="ExternalInput")
    ew3_in = nc.dram_tensor("ew3", [32, 16], FP, kind="ExternalInput")
    cw1_in = nc.dram_tensor("cw1", [16, 32], FP, kind="ExternalInput")
    cw2_in = nc.dram_tensor("cw2", [32, 32], FP, kind="ExternalInput")
    cw3_in = nc.dram_tensor("cw3", [32, 3], FP, kind="ExternalInput")
    nw1t_in = nc.dram_tensor("nw1t", [1, 128], FP, kind="ExternalInput")
    nw1h_in = nc.dram_tensor("nw1h", [64, 128], FP, kind="ExternalInput")
    nw1a_in = nc.dram_tensor("nw1a", [16, 128], FP, kind="ExternalInput")
    nw2_in = nc.dram_tensor("nw2", [128, 128], FP, kind="ExternalInput")
    nw3_in = nc.dram_tensor("nw3", [128, 64], FP, kind="ExternalInput")
    bias_in = nc.dram_tensor("biases", [128, 8], FP, kind="ExternalInput")
    wr_in = nc.dram_tensor("wr3", [3, 64], FP, kind="ExternalInput")
    e3_in = nc.dram_tensor("e3", [3, 19], FP, kind="ExternalInput")
    h_out = nc.dram_tensor("h_new", [64, NPAD], FP, kind="ExternalOutput")
    p_out = nc.dram_tensor("p_new", [3, NPAD], FP, kind="ExternalOutput")
    tabA_out = nc.dram_tensor("tabA", [NPAD, 68], FP, kind="ExternalOutput")
    tabB_out = nc.dram_tensor("tabB", [NPAD, 68], FP, kind="ExternalOutput")

    with tile.TileContext(nc) as tc:
        with tc.tile_pool(name="cst", bufs=1) as cst, \
             tc.tile_pool(name="st", bufs=1) as st, \
             tc.tile_pool(name="g", bufs=2) as gp, \
             tc.tile_pool(name="wk", bufs=2) as wk, \
             tc.tile_pool(name="ps", bufs=2, space="PSUM") as ps, \
             tc.tile_pool(name="psA", bufs=1, space="PSUM") as psA, \
             tc.tile_pool(name="psB", bufs=2, space="PSUM") as psB, \
             tc.tile_pool(name="psG", bufs=1, space="PSUM") as psG, \
             tc.tile_pool(name="dr", bufs=1, space="DRAM") as dr:
            h_t = st.tile([64, NPAD], FP)
            p_t = st.tile([3, NPAD], FP)
            gi_t = cst.tile([128, NCALL], mybir.dt.int32)
            trow_t = cst.tile([1, 512], FP)
            ones_t = cst.tile([1, 128], FP)
            aggvE_d = dr.tile([16, NVV], FP)
            aggvT_d = dr.tile([3, NVV], FP)
            wab0_t = cst.tile([1, 136], FP)
            wabh_t = cst.tile([64, 136], FP)
            wabp_t = cst.tile([3, 136], FP)
            wabq_t = cst.tile([3, 136], FP)
            ew2_t = cst.tile([64, 32], FP)
            ew3_t = cst.tile([32, 16], FP)
            cw1_t = cst.tile([16, 32], FP)
            cw2_t = cst.tile([32, 32], FP)
            cw3_t = cst.tile([32, 3], FP)
            nw1t_t = cst.tile([1, 128], FP)
            nw1h_t = cst.tile([64, 128], FP)
            nw1a_t = cst.tile([16, 128], FP)
            nw2_t = cst.tile([128, 128], FP)
            nw3_t = cst.tile([128, 64], FP)
            bias_t = cst.tile([128, 8], FP)
            wr_t = cst.tile([3, 64], FP)
            e3_t = cst.tile([3, 19], FP)
            ident_t = cst.tile([128, 128], FP)
            aggrE_t = st.tile([16, NPAD], FP)
            aggrT_t = st.tile([3, NPAD], FP)
            for ap, src in [(h_t, h_in), (p_t, p_in), (gi_t, gi_in),
                            (trow_t, trow_in),
                            (wab0_t, wab0_in), (wabh_t, wabh_in),
                            (wabp_t, wabp_in), (wabq_t, wabq_in),
                            (ew2_t, ew2_in), (ew3_t, ew3_in), (cw1_t, cw1_in),
                            (cw2_t, cw2_in), (cw3_t, cw3_in),
                            (nw1t_t, nw1t_in), (nw1h_t, nw1h_in), (nw1a_t, nw1a_in),
                            (nw2_t, nw2_in), (nw3_t, nw3_in), (bias_t, bias_in),
                            (wr_t, wr_in), (e3_t, e3_in)]:
                nc.sync.dma_start(out=ap[:], in_=src[:])
            nc.vector.memset(ones_t[:], 1.0)
            from concourse.masks import make_identity
            make_identity(nc, ident_t[:])

            for w in range(NWIN):
                avw = wk.tile([128, 4 * 68], FP, tag="wD")
                nc.sync.dma_start(out=avw[:], in_=av_in[:, (4 * w) * 68:(4 * w + 4) * 68])
                awp = psA.tile([68, 512], FP, tag="awp")
                for q in range(4):
                    nc.tensor.transpose(out=awp[:, q * 128:(q + 1) * 128],
                                        in_=avw[:, q * 68:(q + 1) * 68],
                                        identity=ident_t[:])
                aw = wk.tile([68, 512], FP, tag="aw")
                nc.vector.tensor_copy(out=aw[:], in_=awp[:])
                pagg = psG.tile([19, 512], FP, tag="agg")
                for s in range(SLOTS):
                    ch = w * SLOTS + s
                    base = ch * 512
                    zbp = psB.tile([68, 512], FP, tag="zbp")
                    gb = gp.tile([128, 4 * 68], FP, tag="gb")
                    for q in range(4):
                        nc.gpsimd.indirect_dma_start(
                            out=gb[:, q * 68:(q + 1) * 68], out_offset=None, in_=bt_in[:],
                            in_offset=bass.IndirectOffsetOnAxis(
                                ap=gi_t[:, ch * 4 + q:ch * 4 + q + 1], axis=0))
                    for q in range(4):
                        nc.tensor.transpose(out=zbp[:, q * 128:(q + 1) * 128],
                                            in_=gb[:, q * 68:(q + 1) * 68], identity=ident_t[:])
                    epc = wk.tile([64, 512], FP, tag="wG")
                    mkc = wk.tile([16, 512], FP, tag="wM")
                    mkt = wk.tile([3, 512], FP, tag="wM2")
                    nc.sync.dma_start(out=epc[:], in_=ep_in[:, base:base + 512])
                    nc.sync.dma_start(out=mkc[:], in_=maskE_in[:, base:base + 512])
                    nc.sync.dma_start(out=mkt[:], in_=maskT_in[:, base:base + 512])
                    pp = wk.tile([3, 512], FP, tag="wA")
                    nc.vector.tensor_tensor(out=pp[:], in0=aw[64:67, :], in1=zbp[64:67, :], op=mybir.AluOpType.mult)
                    pcr = ps.tile([128, 512], FP, tag="pe")
                    nc.tensor.matmul(out=pcr[0:64, :], lhsT=wr_t[:], rhs=pp[:],
                                     start=True, stop=True)
                    z = wk.tile([64, 512], FP, tag="wC")
                    nc.vector.tensor_tensor(out=z[:], in0=aw[0:64, :], in1=zbp[0:64, :], op=mybir.AluOpType.add)
                    nc.vector.tensor_tensor(out=z[:], in0=z[:], in1=epc[:], op=mybir.AluOpType.add)
                    nc.vector.tensor_tensor(out=z[:], in0=z[:], in1=pcr[0:64, :], op=mybir.AluOpType.add)
                    h1 = wk.tile([64, 512], FP, tag="wB")
                    _lrelu(nc, h1[:], z[:], wk, (64, 512))
                    cdf = wk.tile([3, 512], FP, tag="wD")
                    nc.vector.tensor_tensor(out=cdf[:], in0=aw[64:67, :], in1=zbp[64:67, :], op=mybir.AluOpType.subtract)
                    p2 = ps.tile([128, 512], FP, tag="pe")
                    nc.tensor.matmul(out=p2[0:32, :], lhsT=ew2_t[:], rhs=h1[:], start=True, stop=True)
                    h2 = wk.tile([32, 512], FP, tag="wA")
                    nc.vector.tensor_tensor(out=h2[:], in0=p2[0:32, :], in1=bias_t[0:32, 0:1].to_broadcast([32, 512]), op=mybir.AluOpType.add)
                    _lrelu(nc, h2[:], h2[:], wk, (32, 512))
                    p3 = ps.tile([128, 512], FP, tag="pe")
                    nc.tensor.matmul(out=p3[0:16, :], lhsT=ew3_t[:], rhs=h2[:], start=True, stop=True)
                    ef = wk.tile([16, 512], FP, tag="wE")
                    nc.vector.tensor_tensor(out=ef[:], in0=p3[0:16, :], in1=bias_t[0:16, 1:2].to_broadcast([16, 512]), op=mybir.AluOpType.add)
                    pc1 = ps.tile([128, 512], FP, tag="pe")
                    nc.tensor.matmul(out=pc1[0:32, :], lhsT=cw1_t[:], rhs=ef[:], start=True, stop=True)
                    c1 = wk.tile([32, 512], FP, tag="wB")
                    nc.vector.tensor_tensor(out=c1[:], in0=pc1[0:32, :], in1=bias_t[0:32, 2:3].to_broadcast([32, 512]), op=mybir.AluOpType.add)
                    _lrelu(nc, c1[:], c1[:], wk, (32, 512))
                    pc2 = ps.tile([128, 512], FP, tag="pe")
                    nc.tensor.matmul(out=pc2[0:32, :], lhsT=cw2_t[:], rhs=c1[:], start=True, stop=True)
                    c2 = wk.tile([32, 512], FP, tag="wA")
                    nc.vector.tensor_tensor(out=c2[:], in0=pc2[0:32, :], in1=bias_t[0:32, 3:4].to_broadcast([32, 512]), op=mybir.AluOpType.add)
                    _lrelu(nc, c2[:], c2[:], wk, (32, 512))
                    pc3 = ps.tile([128, 512], FP, tag="pe")
                    nc.tensor.matmul(out=pc3[0:3, :], lhsT=cw3_t[:], rhs=c2[:], start=True, stop=True)
                    cm = wk.tile([3, 512], FP, tag="wF")
                    nc.vector.tensor_tensor(out=cm[:], in0=pc3[0:3, :], in1=bias_t[0:3, 4:5].to_broadcast([3, 512]), op=mybir.AluOpType.add)
                    valsE = wk.tile([16, 512], FP, tag="wV")
                    valsT = wk.tile([3, 512], FP, tag="wV2")
                    nc.vector.tensor_tensor(out=valsE[:], in0=ef[:], in1=mkc[:], op=mybir.AluOpType.mult)
                    nc.vector.tensor_tensor(out=valsT[:], in0=cdf[:], in1=cm[:], op=mybir.AluOpType.mult)
                    nc.vector.tensor_tensor(out=valsT[:], in0=valsT[:], in1=mkt[:], op=mybir.AluOpType.mult)
                    nc.tensor.matmul(out=pagg[:], lhsT=ident_t[0:16, 0:19], rhs=valsE[:],
                                     start=(s == 0), stop=False)
                    nc.tensor.matmul(out=pagg[:], lhsT=e3_t[:], rhs=valsT[:],
                                     start=False, stop=(s == SLOTS - 1))
                agw = wk.tile([19, 512], FP, tag="wG")
                nc.vector.tensor_copy(out=agw[:], in_=pagg[:])
                nc.sync.dma_start(out=aggvE_d[:, w * 512:(w + 1) * 512], in_=agw[0:16, :])
                nc.sync.dma_start(out=aggvT_d[:, w * 512:(w + 1) * 512], in_=agw[16:19, :])

            for (dsrc, dst_t, nr) in [(aggvE_d, aggrE_t, 16), (aggvT_d, aggrT_t, 3)]:
                for (k, rstart, vstart, nk) in classes_u:
                    done = 0
                    while done < nk:
                        nsub = min(nk - done, 2048 // k)
                        cmb = st.tile([16, 2048], FP, tag="cmb")
                        nc.sync.dma_start(out=cmb[0:nr, :nsub * k],
                                          in_=dsrc[:, vstart + done * k: vstart + (done + nsub) * k])
                        s3 = cmb[0:nr, :nsub * k].rearrange("p (n k) -> p n k", k=k)
                        rs0 = rstart + done
                        nc.vector.tensor_copy(out=dst_t[:, rs0:rs0 + nsub], in_=s3[:, :, 0])
                        for kk in range(1, k):
                            nc.vector.tensor_tensor(out=dst_t[:, rs0:rs0 + nsub],
                                                    in0=dst_t[:, rs0:rs0 + nsub],
                                                    in1=s3[:, :, kk], op=mybir.AluOpType.add)
                        done += nsub

            hn_t = h_t
            for b in range(NPAD // 512):
                sl = slice(b * 512, (b + 1) * 512)
                pn1 = ps.tile([128, 512], FP, tag="pe")
                nc.tensor.matmul(out=pn1[:], lhsT=nw1t_t[:], rhs=trow_t[:], start=True, stop=False)
                nc.tensor.matmul(out=pn1[:], lhsT=nw1h_t[:], rhs=h_t[:, sl], start=False, stop=False)
                nc.tensor.matmul(out=pn1[:], lhsT=nw1a_t[:], rhs=aggrE_t[:, sl], start=False, stop=True)
                n1 = wk.tile([128, 512], FP, tag="wA")
                nc.vector.tensor_tensor(out=n1[:], in0=pn1[:], in1=bias_t[:, 5:6].to_broadcast([128, 512]), op=mybir.AluOpType.add)
                _lrelu(nc, n1[:], n1[:], wk, (128, 512))
                pn2 = ps.tile([128, 512], FP, tag="pe")
                nc.tensor.matmul(out=pn2[:], lhsT=nw2_t[:], rhs=n1[:], start=True, stop=True)
                n2 = wk.tile([128, 512], FP, tag="wB")
                nc.vector.tensor_tensor(out=n2[:], in0=pn2[:], in1=bias_t[:, 6:7].to_broadcast([128, 512]), op=mybir.AluOpType.add)
                _lrelu(nc, n2[:], n2[:], wk, (128, 512))
                pn3 = ps.tile([128, 512], FP, tag="pe")
                nc.tensor.matmul(out=pn3[0:64, :], lhsT=nw3_t[:], rhs=n2[:], start=True, stop=True)
                nh = wk.tile([64, 512], FP, tag="wC")
                nc.vector.tensor_tensor(out=nh[:], in0=pn3[0:64, :], in1=bias_t[0:64, 7:8].to_broadcast([64, 512]), op=mybir.AluOpType.add)
                nc.scalar.mul(out=nh[:], in_=nh[:], mul=DT)
                nc.vector.tensor_tensor(out=hn_t[:, sl], in0=h_t[:, sl], in1=nh[:], op=mybir.AluOpType.add)
            nc.scalar.mul(out=aggrT_t[:], in_=aggrT_t[:], mul=DT)
            nc.scalar.mul(out=p_t[:], in_=p_t[:], mul=1.0 + DT)
            nc.vector.tensor_tensor(out=p_t[:], in0=p_t[:], in1=aggrT_t[:], op=mybir.AluOpType.add)
            nc.sync.dma_start(out=h_out[:], in_=hn_t[:])
            nc.sync.dma_start(out=p_out[:], in_=p_t[:])
            # tables via decomposed matmuls: rows [ones, h, p, p^2]
            for b in range(NPAD // 128):
                cs = slice(b * 128, (b + 1) * 128)
                psq = wk.tile([3, 128], FP, tag="wF")
                nc.vector.tensor_tensor(out=psq[:], in0=p_t[:, cs], in1=p_t[:, cs], op=mybir.AluOpType.mult)
                for (rhs_sl, outT, tag) in [(slice(0, 68), tabA_out, "tsA"), (slice(68, 136), tabB_out, "tsB")]:
                    pa = ps.tile([128, 68], FP, tag="pe")
                    nc.tensor.matmul(out=pa[:], lhsT=ones_t[:], rhs=wab0_t[:, rhs_sl], start=True, stop=False)
                    nc.tensor.matmul(out=pa[:], lhsT=hn_t[:, cs], rhs=wabh_t[:, rhs_sl], start=False, stop=False)
                    nc.tensor.matmul(out=pa[:], lhsT=p_t[:, cs], rhs=wabp_t[:, rhs_sl], start=False, stop=False)
                    nc.tensor.matmul(out=pa[:], lhsT=psq[:], rhs=wabq_t[:, rhs_sl], start=False, stop=True)
                    sa = wk.tile([128, 68], FP, tag=tag)
                    nc.vector.tensor_copy(out=sa[:], in_=pa[:])
                    nc.sync.dma_start(out=outT[cs, :], in_=sa[:])
    nc.compile()
    return nc


def kernel(**inputs):
    inputs = {k: np.asarray(v) for k, v in inputs.items()}
    eW1, eb1 = inputs["eW1"].astype(np.float32), inputs["eb1"].astype(np.float32)
    We = eW1[131:135]
    ei = inputs["edge_index"].astype(np.int64)
    row, col = ei[0], ei[1]
    # pass 1: uniform class layout
    per_core = []
    nk_all = {}
    for c in range(NC):
        m = (row // NLOC) == c
        rl = row[m] - c * NLOC
        _, vdeg = _vdeg_of(rl)
        cnts = np.bincount(vdeg[:NLOC])
        per_core.append((m, rl))
        for k in range(1, cnts.size):
            if cnts[k]:
                nk_all[k] = max(nk_all.get(k, 0), int(cnts[k]))
    classes_u = []
    rstart = vstart = 0
    for k in sorted(nk_all):
        classes_u.append((k, rstart, vstart, nk_all[k]))
        rstart += nk_all[k]
        vstart += nk_all[k] * k
    assert rstart <= NPAD, rstart
    assert vstart <= NVV, vstart
    cores = []
    ea = inputs["edge_attr"].astype(np.float32)
    for c in range(NC):
        m, rl = per_core[c]
        cores.append(_prep_core(rl, col[m], np.nonzero(m)[0], ea, We, classes_u))
    # translate gather idx to table rows (class-permuted global)
    invs = [cd["inv_order"] for cd in cores]
    for cd in cores:
        g = cd["gidx"]
        co = g // NLOC
        lo = g % NLOC
        grow = np.zeros(ES, np.int64)
        for c2 in range(NC):
            mm = co == c2
            grow[mm] = c2 * NPAD + invs[c2][lo[mm]]
        cd["grow"] = grow.reshape(ES // 128, 128).T.astype(np.int32).copy()
        ar = np.arange(ES)
        virt_of_pos = (ar // (8 * VW)) * VW + (ar % VW)
        cinv = (1.0 / cd["cnt"]).astype(np.float32)[cd["v2r"][virt_of_pos]]
        m19 = np.zeros((19, ES), np.float32)
        m19[0:16] = cd["mask"]
        m19[16:19] = cd["mask"] * cinv
        cd["mask2"] = m19

    times = np.linspace(0.0, 2.0, 4).astype(np.float32)
    embW = inputs["emb_W"].astype(np.float32)
    embB = inputs["emb_b"].astype(np.float32).reshape(64, 1)
    wabs = [np.concatenate(_build_wab(float(t), eW1, eb1), axis=1) for t in times]
    bias = np.zeros((128, 8), np.float32)
    bias[0:32, 0] = inputs["eb2"]; bias[0:16, 1] = inputs["eb3"]
    bias[0:32, 2] = inputs["cb1"]; bias[0:32, 3] = inputs["cb2"]
    bias[0:3, 4] = inputs["cb3"]; bias[:, 5] = inputs["nb1"]
    bias[:, 6] = inputs["nb2"]; bias[0:64, 7] = inputs["nb3"]
    wr = eW1[130].reshape(64, 1).astype(np.float32)
    ones_row = np.ones((1, NPAD), np.float32)

    global _DBG_CLASSES
    _DBG_CLASSES = classes_u
    p0 = build_p0()
    psp = build_ps(classes_u)

    x = inputs["x"].astype(np.float32); pos = inputs["pos"].astype(np.float32)
    in0 = []
    for c in range(NC):
        od = cores[c]["order"]
        xs = np.zeros((NPAD, F), np.float32); xs[:NLOC] = x[c*NLOC:(c+1)*NLOC]
        pp = np.zeros((NPAD, 3), np.float32); pp[:NLOC] = pos[c*NLOC:(c+1)*NLOC]
        in0.append({"x_fm": np.ascontiguousarray(xs[od].T), "pos_fm": np.ascontiguousarray(pp[od].T),
                    "embW": embW, "embB": embB, "wab0": wabs[0][0:1], "wabh": wabs[0][1:65],
                    "wabp": wabs[0][65:68], "wabq": wabs[0][68:71], "ones": ones_row})
    global LAST_EXEC_NS
    _tot = 0
    r0 = bass_utils.run_bass_kernel_spmd(p0, in0, core_ids=list(range(NC)), trace=TRACE)
    if TRACE and r0.exec_time_ns:
        _tot += r0.exec_time_ns
    h_fm = [r0.results[c]["h0"] for c in range(NC)]
    p_fm = [in0[c]["pos_fm"] for c in range(NC)]
    tabA = [r0.results[c]["tabA"] for c in range(NC)]
    tabB = [r0.results[c]["tabB"] for c in range(NC)]

    out = np.zeros((4, N, H), np.float32)
    for c in range(NC):
        inv = cores[c]["inv_order"]
        out[0, c*NLOC:(c+1)*NLOC] = h_fm[c].T[inv[:NLOC]]

    ew2 = inputs["eW2"].astype(np.float32)
    for step in range(3):
        t = float(times[step])
        btab = np.ascontiguousarray(np.concatenate(tabB, axis=0))
        in_s = []
        for c in range(NC):
            cd = cores[c]
            avirt = tabA[c][cd["v2r"]]                  # [NVV, 68]
            avs = np.ascontiguousarray(
                avirt.reshape(NVV // 128, 128, 68).transpose(1, 0, 2).reshape(128, -1))
            in_s.append({
                "h": h_fm[c], "p": p_fm[c], "avirt": avs, "btab": btab,
                "gidx": cd["grow"],
                "maskE": cd["mask2"][0:16], "maskT": cd["mask2"][16:19],
                "eproj": np.ascontiguousarray(cd["eproj"].T),
                "cnti": (1.0 / cd["cnt"]).astype(np.float32).reshape(1, NPAD),
                "trow": np.full((1, 512), t, np.float32),
                "wab0": wabs[step + 1][0:1], "wabh": wabs[step + 1][1:65],
                "wabp": wabs[step + 1][65:68], "wabq": wabs[step + 1][68:71],
                "ew2": ew2, "ew3": inputs["eW3"].astype(np.float32),
                "cw1": inputs["cW1"].astype(np.float32),
                "cw2": inputs["cW2"].astype(np.float32),
                "cw3": inputs["cW3"].astype(np.float32),
                "nw1t": inputs["nW1"][0:1].astype(np.float32),
                "nw1h": inputs["nW1"][1:65].astype(np.float32),
                "nw1a": inputs["nW1"][65:81].astype(np.float32),
                "nw2": inputs["nW2"].astype(np.float32),
                "nw3": inputs["nW3"].astype(np.float32),
                "biases": bias, "wr3": np.tile(-2.0 * wr.T, (3, 1)).copy(),
                "e3": np.eye(19, dtype=np.float32)[16:19].copy(),
            })
        rs = bass_utils.run_bass_kernel_spmd(psp, in_s, core_ids=list(range(NC)), trace=TRACE)
        if TRACE and rs.exec_time_ns:
            _tot += rs.exec_time_ns
        for c in range(NC):
            h_fm[c] = rs.results[c]["h_new"]
            p_fm[c] = rs.results[c]["p_new"]
            tabA[c] = rs.results[c]["tabA"]
            tabB[c] = rs.results[c]["tabB"]
            inv = cores[c]["inv_order"]
            out[step + 1, c*NLOC:(c+1)*NLOC] = h_fm[c].T[inv[:NLOC]]
    LAST_EXEC_NS = _tot if TRACE else None
    return out


# revision 13
# speedup vs baseline: 1.0323x; 1.0323x over previous
"""EGNN continuous ODE on 8 Trainium2 NeuronCores (Bass/Tile).

Nodes sharded 8-way (per-core class-ordered with padding), edges sharded by
row-core into an 8-slot virtual-node grid. Per Euler step one SPMD launch:
B-side rows fetched with 128-row indirect DMA gathers + PE transposes,
edge MLP feature-major on PE, slot aggregation via PSUM-accumulating matmuls,
virtual->real combine, node MLP, and next-step table build. Host glues the
4 launches (global B-table assembly, virtual expansion, permutations).
"""
import sys
sys.path.insert(0, '/opt/trn_rl_repo')
import numpy as np
from concourse import bass, tile, mybir, bass_utils, bacc

H = 64
F = 128
N = 50000
NC = 8
NLOC = 6250
NPAD = 6656          # padded real nodes per core (13*512)
SLOTS = 8
VW = 512
SLOPE = 0.3
NWIN = 32
NVV = NWIN * VW      # 16384 virtual nodes padded
ES = NVV * SLOTS     # 131072 slot stream
NCALL = ES // 128    # 1024 gather calls / step
FP = mybir.dt.float32
DT = float(2.0 / 3.0)
TRACE = False
LAST_EXEC_NS = None


def _vdeg_of(row_l):
    deg = np.bincount(row_l, minlength=NPAD)
    return deg, np.maximum((deg + SLOTS - 1) // SLOTS, 1)


def _prep_core(row_l, col_g, eids, edge_attr, We, classes_u):
    deg, vdeg = _vdeg_of(row_l)
    # build class-ordered real-node permutation under the UNIFORM layout
    order = np.full(NPAD, -1, np.int64)     # class-pos -> orig local id
    by_k = {}
    for i in np.argsort(vdeg[:NLOC], kind="stable"):
        by_k.setdefault(int(vdeg[i]), []).append(int(i))
    pad_pool = list(range(NLOC, NPAD))      # dummy real nodes (deg 0)
    v2r = np.zeros(NVV, np.int64)           # virtual -> class-pos of real node
    for (k, rstart, vstart, nk) in classes_u:
        mine = by_k.get(k, [])
        assert len(mine) <= nk, (k, len(mine), nk)
        for i, orig in enumerate(mine):
            order[rstart + i] = orig
        for i in range(len(mine), nk):
            order[rstart + i] = pad_pool.pop()
        for i in range(nk * k):
            v2r[vstart + i] = rstart + i // k
    # leftover class positions (beyond classes_u coverage) get remaining pads
    for i in range(NPAD):
        if order[i] < 0:
            order[i] = pad_pool.pop()
    inv_order = np.argsort(order)
    # slot assignment for real edges
    first_v = np.zeros(NPAD, np.int64)      # orig local -> first virtual id
    cls_pos_of = inv_order                  # orig local -> class pos
    # first virtual of each class position
    firstv_of_cpos = np.zeros(NPAD, np.int64)
    for (k, rstart, vstart, nk) in classes_u:
        firstv_of_cpos[rstart:rstart + nk] = vstart + np.arange(nk) * k
    first_v = firstv_of_cpos[cls_pos_of]
    srt = np.argsort(row_l, kind="stable")
    rl_s, cg_s, eid_s = row_l[srt], col_g[srt], eids[srt]
    first_e = np.zeros(NPAD + 1, np.int64)
    np.cumsum(deg, out=first_e[1:])
    within = np.arange(rl_s.size) - first_e[rl_s]
    virt = first_v[rl_s] + within // SLOTS
    slot = within % SLOTS
    pos = (virt // VW) * (SLOTS * VW) + slot * VW + (virt % VW)
    gidx = np.zeros(ES, np.int64)
    mask = np.zeros(ES, np.float32)
    ep = np.zeros((ES, H), np.float32)
    gidx[pos] = cg_s
    mask[pos] = 1.0
    ep[pos] = edge_attr[eid_s] @ We
    cnt = np.maximum(deg, 1).astype(np.float32)[order]   # class order
    return dict(order=order, inv_order=inv_order, v2r=v2r,
                gidx=gidx, mask=mask, eproj=ep, cnt=cnt)


def _build_wab(t, eW1, eb1):
    Wa = np.zeros((71, 68), np.float32)
    Wb = np.zeros((71, 68), np.float32)
    Wr = eW1[130]
    Wa[0, :64] = t * eW1[0] + eb1
    Wa[1:65, :64] = eW1[1:65]
    Wa[65:68, 64:67] = np.eye(3)
    Wa[68:71, :64] = Wr
    Wb[0, :64] = t * eW1[65]
    Wb[1:65, :64] = eW1[66:130]
    Wb[65:68, 64:67] = np.eye(3)
    Wb[68:71, :64] = Wr
    return Wa, Wb


def _lrelu(nc, out_ap, in_ap, pool, shape):
    tmp = pool.tile(list(shape), FP, tag="lrt")
    nc.scalar.mul(out=tmp[:], in_=in_ap, mul=SLOPE)
    nc.vector.tensor_tensor(out=out_ap, in0=in_ap, in1=tmp[:], op=mybir.AluOpType.max)


def _build_ext(nc, pool, h_ap, p_ap, ones_ap):
    ext = pool.tile([71, NPAD], FP, tag="ext")
    nc.vector.tensor_copy(out=ext[0:1, :], in_=ones_ap)
    nc.vector.tensor_copy(out=ext[1:65, :], in_=h_ap)
    nc.vector.tensor_copy(out=ext[65:68, :], in_=p_ap)
    nc.vector.tensor_tensor(out=ext[68:71, :], in0=p_ap, in1=p_ap, op=mybir.AluOpType.mult)
    return ext


def _table_mms(nc, sbuf, psum, ext_t, wab_t, outA, outB):
    for b in range(NPAD // 128):
        pa = psum.tile([128, 68], FP, tag="pe")
        lhsT = ext_t[:, b * 128:(b + 1) * 128]
        nc.tensor.matmul(out=pa[:], lhsT=lhsT, rhs=wab_t[:, 0:68], start=True, stop=True)
        sa = sbuf.tile([128, 68], FP, tag="wE")
        nc.vector.tensor_copy(out=sa[:], in_=pa[:])
        nc.sync.dma_start(out=outA[b * 128:(b + 1) * 128, :], in_=sa[:])
        pb = psum.tile([128, 68], FP, tag="pe")
        nc.tensor.matmul(out=pb[:], lhsT=lhsT, rhs=wab_t[:, 68:136], start=True, stop=True)
        sb2 = sbuf.tile([128, 68], FP, tag="wV")
        nc.vector.tensor_copy(out=sb2[:], in_=pb[:])
        nc.sync.dma_start(out=outB[b * 128:(b + 1) * 128, :], in_=sb2[:])


def build_p0():
    nc = bacc.Bacc("TRN2", target_bir_lowering=False, debug=False, num_devices=NC)
    x_in = nc.dram_tensor("x_fm", [128, NPAD], FP, kind="ExternalInput")
    pos_in = nc.dram_tensor("pos_fm", [3, NPAD], FP, kind="ExternalInput")
    embW_in = nc.dram_tensor("embW", [128, 64], FP, kind="ExternalInput")
    embB_in = nc.dram_tensor("embB", [64, 1], FP, kind="ExternalInput")
    wab0_in = nc.dram_tensor("wab0", [1, 136], FP, kind="ExternalInput")
    wabh_in = nc.dram_tensor("wabh", [64, 136], FP, kind="ExternalInput")
    wabp_in = nc.dram_tensor("wabp", [3, 136], FP, kind="ExternalInput")
    wabq_in = nc.dram_tensor("wabq", [3, 136], FP, kind="ExternalInput")
    ones_in = nc.dram_tensor("ones", [1, NPAD], FP, kind="ExternalInput")
    h0_out = nc.dram_tensor("h0", [64, NPAD], FP, kind="ExternalOutput")
    tabA_out = nc.dram_tensor("tabA", [NPAD, 68], FP, kind="ExternalOutput")
    tabB_out = nc.dram_tensor("tabB", [NPAD, 68], FP, kind="ExternalOutput")
    with tile.TileContext(nc) as tc:
        with tc.tile_pool(name="sb", bufs=1) as sbuf, \
             tc.tile_pool(name="ps", bufs=2, space="PSUM") as psum, \
             tc.tile_pool(name="sb2", bufs=2) as sbuf2:
            x_t = sbuf.tile([128, NPAD], FP)
            pos_t = sbuf.tile([3, NPAD], FP)
            embW_t = sbuf.tile([128, 64], FP)
            embB_t = sbuf.tile([64, 1], FP)
            wab0_t = sbuf.tile([1, 136], FP)
            wabh_t = sbuf.tile([64, 136], FP)
            wabp_t = sbuf.tile([3, 136], FP)
            wabq_t = sbuf.tile([3, 136], FP)
            ones_t = sbuf.tile([1, NPAD], FP)
            h0_t = sbuf.tile([64, NPAD], FP)
            for ap, src in [(x_t, x_in), (pos_t, pos_in), (embW_t, embW_in),
                            (embB_t, embB_in), (wab0_t, wab0_in), (wabh_t, wabh_in),
                            (wabp_t, wabp_in), (wabq_t, wabq_in), (ones_t, ones_in)]:
                nc.sync.dma_start(out=ap[:], in_=src[:])
            for b in range(NPAD // 512):
                pe = psum.tile([64, 512], FP, tag="pe")
                nc.tensor.matmul(out=pe[:], lhsT=embW_t[:],
                                 rhs=x_t[:, b * 512:(b + 1) * 512], start=True, stop=True)
                nc.vector.tensor_tensor(out=h0_t[:, b * 512:(b + 1) * 512],
                                        in0=pe[:], in1=embB_t[:].to_broadcast([64, 512]),
                                        op=mybir.AluOpType.add)
            for b in range(NPAD // 128):
                cs = slice(b * 128, (b + 1) * 128)
                psq = sbuf2.tile([3, 128], FP, tag="psq")
                nc.vector.tensor_tensor(out=psq[:], in0=pos_t[:, cs], in1=pos_t[:, cs], op=mybir.AluOpType.mult)
                for (rhs_sl, outT, tag) in [(slice(0, 68), tabA_out, "tsA"), (slice(68, 136), tabB_out, "tsB")]:
                    pa = psum.tile([128, 68], FP, tag="pe2")
                    nc.tensor.matmul(out=pa[:], lhsT=ones_t[:, cs], rhs=wab0_t[:, rhs_sl], start=True, stop=False)
                    nc.tensor.matmul(out=pa[:], lhsT=h0_t[:, cs], rhs=wabh_t[:, rhs_sl], start=False, stop=False)
                    nc.tensor.matmul(out=pa[:], lhsT=pos_t[:, cs], rhs=wabp_t[:, rhs_sl], start=False, stop=False)
                    nc.tensor.matmul(out=pa[:], lhsT=psq[:], rhs=wabq_t[:, rhs_sl], start=False, stop=True)
                    sa = sbuf2.tile([128, 68], FP, tag=tag)
                    nc.vector.tensor_copy(out=sa[:], in_=pa[:])
                    nc.sync.dma_start(out=outT[cs, :], in_=sa[:])
            nc.sync.dma_start(out=h0_out[:], in_=h0_t[:])
    nc.compile()
    return nc


def build_ps(classes_u, nwin_real):
    nc = bacc.Bacc("TRN2", target_bir_lowering=False, debug=False, num_devices=NC)
    h_in = nc.dram_tensor("h", [64, NPAD], FP, kind="ExternalInput")
    p_in = nc.dram_tensor("p", [3, NPAD], FP, kind="ExternalInput")
    av_in = nc.dram_tensor("avirt", [128, (NVV // 128) * 68], FP, kind="ExternalInput")
    bt_in = nc.dram_tensor("btab", [NC * NPAD, 68], FP, kind="ExternalInput")
    gi_in = nc.dram_tensor("gidx", [128, NCALL], mybir.dt.int32, kind="ExternalInput")
    maskE_in = nc.dram_tensor("maskE", [16, ES], FP, kind="ExternalInput")
    maskT_in = nc.dram_tensor("maskT", [3, ES], FP, kind="ExternalInput")
    ep_in = nc.dram_tensor("eproj", [64, ES], FP, kind="ExternalInput")
    trow_in = nc.dram_tensor("trow", [1, 512], FP, kind="ExternalInput")
    wab0_in = nc.dram_tensor("wab0", [1, 136], FP, kind="ExternalInput")
    wabh_in = nc.dram_tensor("wabh", [64, 136], FP, kind="ExternalInput")
    wabp_in = nc.dram_tensor("wabp", [3, 136], FP, kind="ExternalInput")
    wabq_in = nc.dram_tensor("wabq", [3, 136], FP, kind="ExternalInput")
    ew2_in = nc.dram_tensor("ew2", [64, 32], FP, kind="ExternalInput")
    ew3_in = nc.dram_tensor("ew3", [32, 16], FP, kind="ExternalInput")
    cw1_in = nc.dram_tensor("cw1", [16, 32], FP, kind="ExternalInput")
    cw2_in = nc.dram_tensor("cw2", [32, 32], FP, kind="ExternalInput")
    cw3_in = nc.dram_tensor("cw3", [32, 3], FP, kind="ExternalInput")
    nw1t_in = nc.dram_tensor("nw1t", [1, 128], FP, kind="ExternalInput")
    nw1h_in = nc.dram_tensor("nw1h", [64, 128], FP, kind="ExternalInput")
    nw1a_in = nc.dram_tensor("nw1a", [16, 128], FP, kind="ExternalInput")
    nw2_in = nc.dram_tensor("nw2", [128, 128], FP, kind="ExternalInput")
    nw3_in = nc.dram_tensor("nw3", [128, 64], FP, kind="ExternalInput")
    bias_in = nc.dram_tensor("biases", [128, 8], FP, kind="ExternalInput")
    wr_in = nc.dram_tensor("wr3", [3, 64], FP, kind="ExternalInput")
    e3_in = nc.dram_tensor("e3", [3, 19], FP, kind="ExternalInput")
    h_out = nc.dram_tensor("h_new", [64, NPAD], FP, kind="ExternalOutput")
    p_out = nc.dram_tensor("p_new", [3, NPAD], FP, kind="ExternalOutput")
    tabA_out = nc.dram_tensor("tabA", [NPAD, 68], FP, kind="ExternalOutput")
    tabB_out = nc.dram_tensor("tabB", [NPAD, 68], FP, kind="ExternalOutput")

    with tile.TileContext(nc) as tc:
        with tc.tile_pool(name="cst", bufs=1) as cst, \
             tc.tile_pool(name="st", bufs=1) as st, \
             tc.tile_pool(name="g", bufs=2) as gp, \
             tc.tile_pool(name="wk", bufs=3) as wk, \
             tc.tile_pool(name="ps", bufs=4, space="PSUM") as ps, \
             tc.tile_pool(name="psA", bufs=1, space="PSUM") as psA, \
             tc.tile_pool(name="psB", bufs=2, space="PSUM") as psB, \
             tc.tile_pool(name="psG", bufs=1, space="PSUM") as psG, \
             tc.tile_pool(name="dr", bufs=1, space="DRAM") as dr:
            h_t = st.tile([64, NPAD], FP)
            p_t = st.tile([3, NPAD], FP)
            gi_t = cst.tile([128, NCALL], mybir.dt.int32)
            trow_t = cst.tile([1, 512], FP)
            ones_t = cst.tile([1, 128], FP)
            aggvE_d = dr.tile([16, NVV], FP)
            aggvT_d = dr.tile([3, NVV], FP)
            wab0_t = cst.tile([1, 136], FP)
            wabh_t = cst.tile([64, 136], FP)
            wabp_t = cst.tile([3, 136], FP)
            wabq_t = cst.tile([3, 136], FP)
            ew2_t = cst.tile([64, 32], FP)
            ew3_t = cst.tile([32, 16], FP)
            cw1_t = cst.tile([16, 32], FP)
            cw2_t = cst.tile([32, 32], FP)
            cw3_t = cst.tile([32, 3], FP)
            nw1t_t = cst.tile([1, 128], FP)
            nw1h_t = cst.tile([64, 128], FP)
            nw1a_t = cst.tile([16, 128], FP)
            nw2_t = cst.tile([128, 128], FP)
            nw3_t = cst.tile([128, 64], FP)
            bias_t = cst.tile([128, 8], FP)
            wr_t = cst.tile([3, 64], FP)
            e3_t = cst.tile([3, 19], FP)
            ident_t = cst.tile([128, 128], FP)
            aggrE_t = st.tile([16, NPAD], FP)
            aggrT_t = st.tile([3, NPAD], FP)
            for ap, src in [(h_t, h_in), (p_t, p_in), (gi_t, gi_in),
                            (trow_t, trow_in),
                            (wab0_t, wab0_in), (wabh_t, wabh_in),
                            (wabp_t, wabp_in), (wabq_t, wabq_in),
                            (ew2_t, ew2_in), (ew3_t, ew3_in), (cw1_t, cw1_in),
                            (cw2_t, cw2_in), (cw3_t, cw3_in),
                            (nw1t_t, nw1t_in), (nw1h_t, nw1h_in), (nw1a_t, nw1a_in),
                            (nw2_t, nw2_in), (nw3_t, nw3_in), (bias_t, bias_in),
                            (wr_t, wr_in), (e3_t, e3_in)]:
                nc.sync.dma_start(out=ap[:], in_=src[:])
            nc.vector.memset(ones_t[:], 1.0)
            from concourse.masks import make_identity
            make_identity(nc, ident_t[:])

            for w in range(nwin_real):
                avw = wk.tile([128, 4 * 68], FP, tag="wD")
                nc.sync.dma_start(out=avw[:], in_=av_in[:, (4 * w) * 68:(4 * w + 4) * 68])
                awp = psA.tile([68, 512], FP, tag="awp")
                for q in range(4):
                    nc.tensor.transpose(out=awp[:, q * 128:(q + 1) * 128],
                                        in_=avw[:, q * 68:(q + 1) * 68],
                                        identity=ident_t[:])
                aw = wk.tile([68, 512], FP, tag="aw")
                nc.vector.tensor_copy(out=aw[:], in_=awp[:])
                pagg = psG.tile([19, 512], FP, tag="agg")
                for s in range(SLOTS):
                    ch = w * SLOTS + s
                    base = ch * 512
                    zbp = psB.tile([68, 512], FP, tag="zbp")
                    gb = gp.tile([128, 4 * 68], FP, tag="gb")
                    for q in range(4):
                        nc.gpsimd.indirect_dma_start(
                            out=gb[:, q * 68:(q + 1) * 68], out_offset=None, in_=bt_in[:],
                            in_offset=bass.IndirectOffsetOnAxis(
                                ap=gi_t[:, ch * 4 + q:ch * 4 + q + 1], axis=0))
                    for q in range(4):
                        nc.tensor.transpose(out=zbp[:, q * 128:(q + 1) * 128],
                                            in_=gb[:, q * 68:(q + 1) * 68], identity=ident_t[:])
                    epc = wk.tile([64, 512], FP, tag="wG")
                    mkc = wk.tile([16, 512], FP, tag="wM")
                    mkt = wk.tile([3, 512], FP, tag="wM2")
                    nc.sync.dma_start(out=epc[:], in_=ep_in[:, base:base + 512])
                    nc.sync.dma_start(out=mkc[:], in_=maskE_in[:, base:base + 512])
                    nc.sync.dma_start(out=mkt[:], in_=maskT_in[:, base:base + 512])
                    pp = wk.tile([3, 512], FP, tag="wA")
                    nc.vector.tensor_tensor(out=pp[:], in0=aw[64:67, :], in1=zbp[64:67, :], op=mybir.AluOpType.mult)
                    pcr = ps.tile([128, 512], FP, tag="pe")
                    nc.tensor.matmul(out=pcr[0:64, :], lhsT=wr_t[:], rhs=pp[:],
                                     start=True, stop=True)
                    z = wk.tile([64, 512], FP, tag="wC")
                    nc.vector.tensor_tensor(out=z[:], in0=aw[0:64, :], in1=zbp[0:64, :], op=mybir.AluOpType.add)
                    nc.vector.tensor_tensor(out=z[:], in0=z[:], in1=epc[:], op=mybir.AluOpType.add)
                    nc.vector.tensor_tensor(out=z[:], in0=z[:], in1=pcr[0:64, :], op=mybir.AluOpType.add)
                    h1 = wk.tile([64, 512], FP, tag="wB")
                    _lrelu(nc, h1[:], z[:], wk, (64, 512))
                    cdf = wk.tile([3, 512], FP, tag="wD")
                    nc.vector.tensor_tensor(out=cdf[:], in0=aw[64:67, :], in1=zbp[64:67, :], op=mybir.AluOpType.subtract)
                    p2 = ps.tile([128, 512], FP, tag="pe")
                    nc.tensor.matmul(out=p2[0:32, :], lhsT=ew2_t[:], rhs=h1[:], start=True, stop=True)
                    h2 = wk.tile([32, 512], FP, tag="wA")
                    nc.vector.tensor_tensor(out=h2[:], in0=p2[0:32, :], in1=bias_t[0:32, 0:1].to_broadcast([32, 512]), op=mybir.AluOpType.add)
                    _lrelu(nc, h2[:], h2[:], wk, (32, 512))
                    p3 = ps.tile([128, 512], FP, tag="pe")
                    nc.tensor.matmul(out=p3[0:16, :], lhsT=ew3_t[:], rhs=h2[:], start=True, stop=True)
                    ef = wk.tile([16, 512], FP, tag="wE")
                    nc.vector.tensor_tensor(out=ef[:], in0=p3[0:16, :], in1=bias_t[0:16, 1:2].to_broadcast([16, 512]), op=mybir.AluOpType.add)
                    pc1 = ps.tile([128, 512], FP, tag="pe")
                    nc.tensor.matmul(out=pc1[0:32, :], lhsT=cw1_t[:], rhs=ef[:], start=True, stop=True)
                    c1 = wk.tile([32, 512], FP, tag="wB")
                    nc.vector.tensor_tensor(out=c1[:], in0=pc1[0:32, :], in1=bias_t[0:32, 2:3].to_broadcast([32, 512]), op=mybir.AluOpType.add)
                    _lrelu(nc, c1[:], c1[:], wk, (32, 512))
                    pc2 = ps.tile([128, 512], FP, tag="pe")
                    nc.tensor.matmul(out=pc2[0:32, :], lhsT=cw2_t[:], rhs=c1[:], start=True, stop=True)
                    c2 = wk.tile([32, 512], FP, tag="wA")
                    nc.vector.tensor_tensor(out=c2[:], in0=pc2[0:32, :], in1=bias_t[0:32, 3:4].to_broadcast([32, 512]), op=mybir.AluOpType.add)
                    _lrelu(nc, c2[:], c2[:], wk, (32, 512))
                    pc3 = ps.tile([128, 512], FP, tag="pe")
                    nc.tensor.matmul(out=pc3[0:3, :], lhsT=cw3_t[:], rhs=c2[:], start=True, stop=True)
                    cm = wk.tile([3, 512], FP, tag="wF")
                    nc.vector.tensor_tensor(out=cm[:], in0=pc3[0:3, :], in1=bias_t[0:3, 4:5].to_broadcast([3, 512]), op=mybir.AluOpType.add)
                    valsE = wk.tile([16, 512], FP, tag="wV")
                    valsT = wk.tile([3, 512], FP, tag="wV2")
                    nc.vector.tensor_tensor(out=valsE[:], in0=ef[:], in1=mkc[:], op=mybir.AluOpType.mult)
                    nc.vector.tensor_tensor(out=valsT[:], in0=cdf[:], in1=cm[:], op=mybir.AluOpType.mult)
                    nc.vector.tensor_tensor(out=valsT[:], in0=valsT[:], in1=mkt[:], op=mybir.AluOpType.mult)
                    nc.tensor.matmul(out=pagg[:], lhsT=ident_t[0:16, 0:19], rhs=valsE[:],
                                     start=(s == 0), stop=False)
                    nc.tensor.matmul(out=pagg[:], lhsT=e3_t[:], rhs=valsT[:],
                                     start=False, stop=(s == SLOTS - 1))
                agw = wk.tile([19, 512], FP, tag="wG")
                nc.vector.tensor_copy(out=agw[:], in_=pagg[:])
                nc.sync.dma_start(out=aggvE_d[:, w * 512:(w + 1) * 512], in_=agw[0:16, :])
                nc.sync.dma_start(out=aggvT_d[:, w * 512:(w + 1) * 512], in_=agw[16:19, :])

            for (dsrc, dst_t, nr) in [(aggvE_d, aggrE_t, 16), (aggvT_d, aggrT_t, 3)]:
                for (k, rstart, vstart, nk) in classes_u:
                    done = 0
                    while done < nk:
                        nsub = min(nk - done, 2048 // k)
                        cmb = st.tile([16, 2048], FP, tag="cmb")
                        nc.sync.dma_start(out=cmb[0:nr, :nsub * k],
                                          in_=dsrc[:, vstart + done * k: vstart + (done + nsub) * k])
                        s3 = cmb[0:nr, :nsub * k].rearrange("p (n k) -> p n k", k=k)
                        rs0 = rstart + done
                        nc.vector.tensor_copy(out=dst_t[:, rs0:rs0 + nsub], in_=s3[:, :, 0])
                        for kk in range(1, k):
                            nc.vector.tensor_tensor(out=dst_t[:, rs0:rs0 + nsub],
                                                    in0=dst_t[:, rs0:rs0 + nsub],
                                                    in1=s3[:, :, kk], op=mybir.AluOpType.add)
                        done += nsub

            hn_t = h_t
            for b in range(NPAD // 512):
                sl = slice(b * 512, (b + 1) * 512)
                pn1 = ps.tile([128, 512], FP, tag="pe")
                nc.tensor.matmul(out=pn1[:], lhsT=nw1t_t[:], rhs=trow_t[:], start=True, stop=False)
                nc.tensor.matmul(out=pn1[:], lhsT=nw1h_t[:], rhs=h_t[:, sl], start=False, stop=False)
                nc.tensor.matmul(out=pn1[:], lhsT=nw1a_t[:], rhs=aggrE_t[:, sl], start=False, stop=True)
                n1 = wk.tile([128, 512], FP, tag="wA")
                nc.vector.tensor_tensor(out=n1[:], in0=pn1[:], in1=bias_t[:, 5:6].to_broadcast([128, 512]), op=mybir.AluOpType.add)
                _lrelu(nc, n1[:], n1[:], wk, (128, 512))
                pn2 = ps.tile([128, 512], FP, tag="pe")
                nc.tensor.matmul(out=pn2[:], lhsT=nw2_t[:], rhs=n1[:], start=True, stop=True)
                n2 = wk.tile([128, 512], FP, tag="wB")
                nc.vector.tensor_tensor(out=n2[:], in0=pn2[:], in1=bias_t[:, 6:7].to_broadcast([128, 512]), op=mybir.AluOpType.add)
                _lrelu(nc, n2[:], n2[:], wk, (128, 512))
                pn3 = ps.tile([128, 512], FP, tag="pe")
                nc.tensor.matmul(out=pn3[0:64, :], lhsT=nw3_t[:], rhs=n2[:], start=True, stop=True)
                nh = wk.tile([64, 512], FP, tag="wC")
                nc.vector.tensor_tensor(out=nh[:], in0=pn3[0:64, :], in1=bias_t[0:64, 7:8].to_broadcast([64, 512]), op=mybir.AluOpType.add)
                nc.scalar.mul(out=nh[:], in_=nh[:], mul=DT)
                nc.vector.tensor_tensor(out=hn_t[:, sl], in0=h_t[:, sl], in1=nh[:], op=mybir.AluOpType.add)
            nc.scalar.mul(out=aggrT_t[:], in_=aggrT_t[:], mul=DT)
            nc.scalar.mul(out=p_t[:], in_=p_t[:], mul=1.0 + DT)
            nc.vector.tensor_tensor(out=p_t[:], in0=p_t[:], in1=aggrT_t[:], op=mybir.AluOpType.add)
            nc.sync.dma_start(out=h_out[:], in_=hn_t[:])
            nc.sync.dma_start(out=p_out[:], in_=p_t[:])
            # tables via decomposed matmuls: rows [ones, h, p, p^2]
            for b in range(NPAD // 128):
                cs = slice(b * 128, (b + 1) * 128)
                psq = wk.tile([3, 128], FP, tag="wF")
                nc.vector.tensor_tensor(out=psq[:], in0=p_t[:, cs], in1=p_t[:, cs], op=mybir.AluOpType.mult)
                for (rhs_sl, outT, tag) in [(slice(0, 68), tabA_out, "tsA"), (slice(68, 136), tabB_out, "tsB")]:
                    pa = ps.tile([128, 68], FP, tag="pe")
                    nc.tensor.matmul(out=pa[:], lhsT=ones_t[:], rhs=wab0_t[:, rhs_sl], start=True, stop=False)
                    nc.tensor.matmul(out=pa[:], lhsT=hn_t[:, cs], rhs=wabh_t[:, rhs_sl], start=False, stop=False)
                    nc.tensor.matmul(out=pa[:], lhsT=p_t[:, cs], rhs=wabp_t[:, rhs_sl], start=False, stop=False)
                    nc.tensor.matmul(out=pa[:], lhsT=psq[:], rhs=wabq_t[:, rhs_sl], start=False, stop=True)
                    sa = wk.tile([128, 68], FP, tag=tag)
                    nc.vector.tensor_copy(out=sa[:], in_=pa[:])
                    nc.sync.dma_start(out=outT[cs, :], in_=sa[:])
    nc.compile()
    return nc


def kernel(**inputs):
    inputs = {k: np.asarray(v) for k, v in inputs.items()}
    eW1, eb1 = inputs["eW1"].astype(np.float32), inputs["eb1"].astype(np.float32)
    We = eW1[131:135]
    ei = inputs["edge_index"].astype(np.int64)
    row, col = ei[0], ei[1]
    # pass 1: uniform class layout
    per_core = []
    nk_all = {}
    for c in range(NC):
        m = (row // NLOC) == c
        rl = row[m] - c * NLOC
        _, vdeg = _vdeg_of(rl)
        cnts = np.bincount(vdeg[:NLOC])
        per_core.append((m, rl))
        for k in range(1, cnts.size):
            if cnts[k]:
                nk_all[k] = max(nk_all.get(k, 0), int(cnts[k]))
    classes_u = []
    rstart = vstart = 0
    for k in sorted(nk_all):
        classes_u.append((k, rstart, vstart, nk_all[k]))
        rstart += nk_all[k]
        vstart += nk_all[k] * k
    assert rstart <= NPAD, rstart
    assert vstart <= NVV, vstart
    cores = []
    ea = inputs["edge_attr"].astype(np.float32)
    for c in range(NC):
        m, rl = per_core[c]
        cores.append(_prep_core(rl, col[m], np.nonzero(m)[0], ea, We, classes_u))
    # translate gather idx to table rows (class-permuted global)
    invs = [cd["inv_order"] for cd in cores]
    for cd in cores:
        g = cd["gidx"]
        co = g // NLOC
        lo = g % NLOC
        grow = np.zeros(ES, np.int64)
        for c2 in range(NC):
            mm = co == c2
            grow[mm] = c2 * NPAD + invs[c2][lo[mm]]
        cd["grow"] = grow.reshape(ES // 128, 128).T.astype(np.int32).copy()
        ar = np.arange(ES)
        virt_of_pos = (ar // (8 * VW)) * VW + (ar % VW)
        cinv = (1.0 / cd["cnt"]).astype(np.float32)[cd["v2r"][virt_of_pos]]
        m19 = np.zeros((19, ES), np.float32)
        m19[0:16] = cd["mask"]
        m19[16:19] = cd["mask"] * cinv
        cd["mask2"] = m19

    times = np.linspace(0.0, 2.0, 4).astype(np.float32)
    embW = inputs["emb_W"].astype(np.float32)
    embB = inputs["emb_b"].astype(np.float32).reshape(64, 1)
    wabs = [np.concatenate(_build_wab(float(t), eW1, eb1), axis=1) for t in times]
    bias = np.zeros((128, 8), np.float32)
    bias[0:32, 0] = inputs["eb2"]; bias[0:16, 1] = inputs["eb3"]
    bias[0:32, 2] = inputs["cb1"]; bias[0:32, 3] = inputs["cb2"]
    bias[0:3, 4] = inputs["cb3"]; bias[:, 5] = inputs["nb1"]
    bias[:, 6] = inputs["nb2"]; bias[0:64, 7] = inputs["nb3"]
    wr = eW1[130].reshape(64, 1).astype(np.float32)
    ones_row = np.ones((1, NPAD), np.float32)

    global _DBG_CLASSES
    _DBG_CLASSES = classes_u
    p0 = build_p0()
    nv_used = max(vs + k * nk for (k, _, vs, nk) in classes_u)
    nwin_real = (nv_used + VW - 1) // VW
    psp = build_ps(classes_u, nwin_real)

    x = inputs["x"].astype(np.float32); pos = inputs["pos"].astype(np.float32)
    in0 = []
    for c in range(NC):
        od = cores[c]["order"]
        xs = np.zeros((NPAD, F), np.float32); xs[:NLOC] = x[c*NLOC:(c+1)*NLOC]
        pp = np.zeros((NPAD, 3), np.float32); pp[:NLOC] = pos[c*NLOC:(c+1)*NLOC]
        in0.append({"x_fm": np.ascontiguousarray(xs[od].T), "pos_fm": np.ascontiguousarray(pp[od].T),
                    "embW": embW, "embB": embB, "wab0": wabs[0][0:1], "wabh": wabs[0][1:65],
                    "wabp": wabs[0][65:68], "wabq": wabs[0][68:71], "ones": ones_row})
    global LAST_EXEC_NS
    _tot = 0
    r0 = bass_utils.run_bass_kernel_spmd(p0, in0, core_ids=list(range(NC)), trace=TRACE)
    if TRACE and r0.exec_time_ns:
        _tot += r0.exec_time_ns
    h_fm = [r0.results[c]["h0"] for c in range(NC)]
    p_fm = [in0[c]["pos_fm"] for c in range(NC)]
    tabA = [r0.results[c]["tabA"] for c in range(NC)]
    tabB = [r0.results[c]["tabB"] for c in range(NC)]

    out = np.zeros((4, N, H), np.float32)
    for c in range(NC):
        inv = cores[c]["inv_order"]
        out[0, c*NLOC:(c+1)*NLOC] = h_fm[c].T[inv[:NLOC]]

    ew2 = inputs["eW2"].astype(np.float32)
    for step in range(3):
        t = float(times[step])
        btab = np.ascontiguousarray(np.concatenate(tabB, axis=0))
        in_s = []
        for c in range(NC):
            cd = cores[c]
            avirt = tabA[c][cd["v2r"]]                  # [NVV, 68]
            avs = np.ascontiguousarray(
                avirt.reshape(NVV // 128, 128, 68).transpose(1, 0, 2).reshape(128, -1))
            in_s.append({
                "h": h_fm[c], "p": p_fm[c], "avirt": avs, "btab": btab,
                "gidx": cd["grow"],
                "maskE": cd["mask2"][0:16], "maskT": cd["mask2"][16:19],
                "eproj": np.ascontiguousarray(cd["eproj"].T),
                "cnti": (1.0 / cd["cnt"]).astype(np.float32).reshape(1, NPAD),
                "trow": np.full((1, 512), t, np.float32),
                "wab0": wabs[step + 1][0:1], "wabh": wabs[step + 1][1:65],
                "wabp": wabs[step + 1][65:68], "wabq": wabs[step + 1][68:71],
                "ew2": ew2, "ew3": inputs["eW3"].astype(np.float32),
                "cw1": inputs["cW1"].astype(np.float32),
                "cw2": inputs["cW2"].astype(np.float32),
                "cw3": inputs["cW3"].astype(np.float32),
                "nw1t": inputs["nW1"][0:1].astype(np.float32),
                "nw1h": inputs["nW1"][1:65].astype(np.float32),
                "nw1a": inputs["nW1"][65:81].astype(np.float32),
                "nw2": inputs["nW2"].astype(np.float32),
                "nw3": inputs["nW3"].astype(np.float32),
                "biases": bias, "wr3": np.tile(-2.0 * wr.T, (3, 1)).copy(),
                "e3": np.eye(19, dtype=np.float32)[16:19].copy(),
            })
        rs = bass_utils.run_bass_kernel_spmd(psp, in_s, core_ids=list(range(NC)), trace=TRACE)
        if TRACE and rs.exec_time_ns:
            _tot += rs.exec_time_ns
        for c in range(NC):
            h_fm[c] = rs.results[c]["h_new"]
            p_fm[c] = rs.results[c]["p_new"]
            tabA[c] = rs.results[c]["tabA"]
            tabB[c] = rs.results[c]["tabB"]
            inv = cores[c]["inv_order"]
            out[step + 1, c*NLOC:(c+1)*NLOC] = h_fm[c].T[inv[:NLOC]]
    LAST_EXEC_NS = _tot if TRACE else None
    return out


# revision 14
# speedup vs baseline: 1.0358x; 1.0033x over previous
"""EGNN continuous ODE on 8 Trainium2 NeuronCores (Bass/Tile).

Nodes sharded 8-way (per-core class-ordered with padding), edges sharded by
row-core into an 8-slot virtual-node grid. Per Euler step one SPMD launch:
B-side rows fetched with 128-row indirect DMA gathers + PE transposes,
edge MLP feature-major on PE, slot aggregation via PSUM-accumulating matmuls,
virtual->real combine, node MLP, and next-step table build. Host glues the
4 launches (global B-table assembly, virtual expansion, permutations).
"""
import sys
sys.path.insert(0, '/opt/trn_rl_repo')
import numpy as np
from concourse import bass, tile, mybir, bass_utils, bacc

H = 64
F = 128
N = 50000
NC = 8
NLOC = 6250
NPAD = 6656          # padded real nodes per core (13*512)
SLOTS = 8
VW = 512
SLOPE = 0.3
NWIN = 32
NVV = NWIN * VW      # 16384 virtual nodes padded
ES = NVV * SLOTS     # 131072 slot stream
NCALL = ES // 128    # 1024 gather calls / step
FP = mybir.dt.float32
DT = float(2.0 / 3.0)
TRACE = False
LAST_EXEC_NS = None


def _vdeg_of(row_l):
    deg = np.bincount(row_l, minlength=NPAD)
    return deg, np.maximum((deg + SLOTS - 1) // SLOTS, 1)


def _prep_core(row_l, col_g, eids, edge_attr, We, classes_u):
    deg, vdeg = _vdeg_of(row_l)
    # build class-ordered real-node permutation under the UNIFORM layout
    order = np.full(NPAD, -1, np.int64)     # class-pos -> orig local id
    by_k = {}
    for i in np.argsort(vdeg[:NLOC], kind="stable"):
        by_k.setdefault(int(vdeg[i]), []).append(int(i))
    pad_pool = list(range(NLOC, NPAD))      # dummy real nodes (deg 0)
    v2r = np.zeros(NVV, np.int64)           # virtual -> class-pos of real node
    for (k, rstart, vstart, nk) in classes_u:
        mine = by_k.get(k, [])
        assert len(mine) <= nk, (k, len(mine), nk)
        for i, orig in enumerate(mine):
            order[rstart + i] = orig
        for i in range(len(mine), nk):
            order[rstart + i] = pad_pool.pop()
        for i in range(nk * k):
            v2r[vstart + i] = rstart + i // k
    # leftover class positions (beyond classes_u coverage) get remaining pads
    for i in range(NPAD):
        if order[i] < 0:
            order[i] = pad_pool.pop()
    inv_order = np.argsort(order)
    # slot assignment for real edges
    first_v = np.zeros(NPAD, np.int64)      # orig local -> first virtual id
    cls_pos_of = inv_order                  # orig local -> class pos
    # first virtual of each class position
    firstv_of_cpos = np.zeros(NPAD, np.int64)
    for (k, rstart, vstart, nk) in classes_u:
        firstv_of_cpos[rstart:rstart + nk] = vstart + np.arange(nk) * k
    first_v = firstv_of_cpos[cls_pos_of]
    srt = np.argsort(row_l, kind="stable")
    rl_s, cg_s, eid_s = row_l[srt], col_g[srt], eids[srt]
    first_e = np.zeros(NPAD + 1, np.int64)
    np.cumsum(deg, out=first_e[1:])
    within = np.arange(rl_s.size) - first_e[rl_s]
    virt = first_v[rl_s] + within // SLOTS
    slot = within % SLOTS
    pos = (virt // VW) * (SLOTS * VW) + slot * VW + (virt % VW)
    gidx = np.zeros(ES, np.int64)
    mask = np.zeros(ES, np.float32)
    ep = np.zeros((ES, H), np.float32)
    gidx[pos] = cg_s
    mask[pos] = 1.0
    ep[pos] = edge_attr[eid_s] @ We
    cnt = np.maximum(deg, 1).astype(np.float32)[order]   # class order
    return dict(order=order, inv_order=inv_order, v2r=v2r,
                gidx=gidx, mask=mask, eproj=ep, cnt=cnt)


def _build_wab(t, eW1, eb1):
    Wa = np.zeros((71, 68), np.float32)
    Wb = np.zeros((71, 68), np.float32)
    Wr = eW1[130]
    Wa[0, :64] = t * eW1[0] + eb1
    Wa[1:65, :64] = eW1[1:65]
    Wa[65:68, 64:67] = np.eye(3)
    Wa[68:71, :64] = Wr
    Wb[0, :64] = t * eW1[65]
    Wb[1:65, :64] = eW1[66:130]
    Wb[65:68, 64:67] = np.eye(3)
    Wb[68:71, :64] = Wr
    return Wa, Wb


def _lrelu(nc, out_ap, in_ap, pool, shape):
    tmp = pool.tile(list(shape), FP, tag="lrt")
    nc.scalar.mul(out=tmp[:], in_=in_ap, mul=SLOPE)
    nc.vector.tensor_tensor(out=out_ap, in0=in_ap, in1=tmp[:], op=mybir.AluOpType.max)


def _build_ext(nc, pool, h_ap, p_ap, ones_ap):
    ext = pool.tile([71, NPAD], FP, tag="ext")
    nc.vector.tensor_copy(out=ext[0:1, :], in_=ones_ap)
    nc.vector.tensor_copy(out=ext[1:65, :], in_=h_ap)
    nc.vector.tensor_copy(out=ext[65:68, :], in_=p_ap)
    nc.vector.tensor_tensor(out=ext[68:71, :], in0=p_ap, in1=p_ap, op=mybir.AluOpType.mult)
    return ext


def _table_mms(nc, sbuf, psum, ext_t, wab_t, outA, outB):
    for b in range(NPAD // 128):
        pa = psum.tile([128, 68], FP, tag="pe")
        lhsT = ext_t[:, b * 128:(b + 1) * 128]
        nc.tensor.matmul(out=pa[:], lhsT=lhsT, rhs=wab_t[:, 0:68], start=True, stop=True)
        sa = sbuf.tile([128, 68], FP, tag="wE")
        nc.vector.tensor_copy(out=sa[:], in_=pa[:])
        nc.sync.dma_start(out=outA[b * 128:(b + 1) * 128, :], in_=sa[:])
        pb = psum.tile([128, 68], FP, tag="pe")
        nc.tensor.matmul(out=pb[:], lhsT=lhsT, rhs=wab_t[:, 68:136], start=True, stop=True)
        sb2 = sbuf.tile([128, 68], FP, tag="wV")
        nc.vector.tensor_copy(out=sb2[:], in_=pb[:])
        nc.sync.dma_start(out=outB[b * 128:(b + 1) * 128, :], in_=sb2[:])


def build_p0():
    nc = bacc.Bacc("TRN2", target_bir_lowering=False, debug=False, num_devices=NC)
    x_in = nc.dram_tensor("x_fm", [128, NPAD], FP, kind="ExternalInput")
    pos_in = nc.dram_tensor("pos_fm", [3, NPAD], FP, kind="ExternalInput")
    embW_in = nc.dram_tensor("embW", [128, 64], FP, kind="ExternalInput")
    embB_in = nc.dram_tensor("embB", [64, 1], FP, kind="ExternalInput")
    wab0_in = nc.dram_tensor("wab0", [1, 136], FP, kind="ExternalInput")
    wabh_in = nc.dram_tensor("wabh", [64, 136], FP, kind="ExternalInput")
    wabp_in = nc.dram_tensor("wabp", [3, 136], FP, kind="ExternalInput")
    wabq_in = nc.dram_tensor("wabq", [3, 136], FP, kind="ExternalInput")
    ones_in = nc.dram_tensor("ones", [1, NPAD], FP, kind="ExternalInput")
    h0_out = nc.dram_tensor("h0", [64, NPAD], FP, kind="ExternalOutput")
    tabA_out = nc.dram_tensor("tabA", [NPAD, 68], FP, kind="ExternalOutput")
    tabB_out = nc.dram_tensor("tabB", [NPAD, 68], FP, kind="ExternalOutput")
    with tile.TileContext(nc) as tc:
        with tc.tile_pool(name="sb", bufs=1) as sbuf, \
             tc.tile_pool(name="ps", bufs=2, space="PSUM") as psum, \
             tc.tile_pool(name="sb2", bufs=2) as sbuf2:
            x_t = sbuf.tile([128, NPAD], FP)
            pos_t = sbuf.tile([3, NPAD], FP)
            embW_t = sbuf.tile([128, 64], FP)
            embB_t = sbuf.tile([64, 1], FP)
            wab0_t = sbuf.tile([1, 136], FP)
            wabh_t = sbuf.tile([64, 136], FP)
            wabp_t = sbuf.tile([3, 136], FP)
            wabq_t = sbuf.tile([3, 136], FP)
            ones_t = sbuf.tile([1, NPAD], FP)
            h0_t = sbuf.tile([64, NPAD], FP)
            for ap, src in [(x_t, x_in), (pos_t, pos_in), (embW_t, embW_in),
                            (embB_t, embB_in), (wab0_t, wab0_in), (wabh_t, wabh_in),
                            (wabp_t, wabp_in), (wabq_t, wabq_in), (ones_t, ones_in)]:
                nc.sync.dma_start(out=ap[:], in_=src[:])
            for b in range(NPAD // 512):
                pe = psum.tile([64, 512], FP, tag="pe")
                nc.tensor.matmul(out=pe[:], lhsT=embW_t[:],
                                 rhs=x_t[:, b * 512:(b + 1) * 512], start=True, stop=True)
                nc.vector.tensor_tensor(out=h0_t[:, b * 512:(b + 1) * 512],
                                        in0=pe[:], in1=embB_t[:].to_broadcast([64, 512]),
                                        op=mybir.AluOpType.add)
            for b in range(NPAD // 128):
                cs = slice(b * 128, (b + 1) * 128)
                psq = sbuf2.tile([3, 128], FP, tag="psq")
                nc.vector.tensor_tensor(out=psq[:], in0=pos_t[:, cs], in1=pos_t[:, cs], op=mybir.AluOpType.mult)
                for (rhs_sl, outT, tag) in [(slice(0, 68), tabA_out, "tsA"), (slice(68, 136), tabB_out, "tsB")]:
                    pa = psum.tile([128, 68], FP, tag="pe2")
                    nc.tensor.matmul(out=pa[:], lhsT=ones_t[:, cs], rhs=wab0_t[:, rhs_sl], start=True, stop=False)
                    nc.tensor.matmul(out=pa[:], lhsT=h0_t[:, cs], rhs=wabh_t[:, rhs_sl], start=False, stop=False)
                    nc.tensor.matmul(out=pa[:], lhsT=pos_t[:, cs], rhs=wabp_t[:, rhs_sl], start=False, stop=False)
                    nc.tensor.matmul(out=pa[:], lhsT=psq[:], rhs=wabq_t[:, rhs_sl], start=False, stop=True)
                    sa = sbuf2.tile([128, 68], FP, tag=tag)
                    nc.vector.tensor_copy(out=sa[:], in_=pa[:])
                    nc.sync.dma_start(out=outT[cs, :], in_=sa[:])
            nc.sync.dma_start(out=h0_out[:], in_=h0_t[:])
    nc.compile()
    return nc


def build_ps(classes_u, nwin_real):
    nc = bacc.Bacc("TRN2", target_bir_lowering=False, debug=False, num_devices=NC)
    h_in = nc.dram_tensor("h", [64, NPAD], FP, kind="ExternalInput")
    p_in = nc.dram_tensor("p", [3, NPAD], FP, kind="ExternalInput")
    av_in = nc.dram_tensor("avirt", [128, (NVV // 128) * 68], FP, kind="ExternalInput")
    bt_in = nc.dram_tensor("btab", [NC * NPAD, 68], FP, kind="ExternalInput")
    gi_in = nc.dram_tensor("gidx", [128, NCALL], mybir.dt.int32, kind="ExternalInput")
    maskE_in = nc.dram_tensor("maskE", [16, ES], FP, kind="ExternalInput")
    maskT_in = nc.dram_tensor("maskT", [3, ES], FP, kind="ExternalInput")
    ep_in = nc.dram_tensor("eproj", [64, ES], FP, kind="ExternalInput")
    trow_in = nc.dram_tensor("trow", [1, 512], FP, kind="ExternalInput")
    wab0_in = nc.dram_tensor("wab0", [1, 136], FP, kind="ExternalInput")
    wabh_in = nc.dram_tensor("wabh", [64, 136], FP, kind="ExternalInput")
    wabp_in = nc.dram_tensor("wabp", [3, 136], FP, kind="ExternalInput")
    wabq_in = nc.dram_tensor("wabq", [3, 136], FP, kind="ExternalInput")
    ew2_in = nc.dram_tensor("ew2", [64, 32], FP, kind="ExternalInput")
    ew3_in = nc.dram_tensor("ew3", [32, 16], FP, kind="ExternalInput")
    cw1_in = nc.dram_tensor("cw1", [16, 32], FP, kind="ExternalInput")
    cw2_in = nc.dram_tensor("cw2", [32, 32], FP, kind="ExternalInput")
    cw3_in = nc.dram_tensor("cw3", [32, 3], FP, kind="ExternalInput")
    nw1t_in = nc.dram_tensor("nw1t", [1, 128], FP, kind="ExternalInput")
    nw1h_in = nc.dram_tensor("nw1h", [64, 128], FP, kind="ExternalInput")
    nw1a_in = nc.dram_tensor("nw1a", [16, 128], FP, kind="ExternalInput")
    nw2_in = nc.dram_tensor("nw2", [128, 128], FP, kind="ExternalInput")
    nw3_in = nc.dram_tensor("nw3", [128, 64], FP, kind="ExternalInput")
    bias_in = nc.dram_tensor("biases", [128, 8], FP, kind="ExternalInput")
    wr_in = nc.dram_tensor("wr3", [3, 64], FP, kind="ExternalInput")
    e3_in = nc.dram_tensor("e3", [3, 19], FP, kind="ExternalInput")
    h_out = nc.dram_tensor("h_new", [64, NPAD], FP, kind="ExternalOutput")
    p_out = nc.dram_tensor("p_new", [3, NPAD], FP, kind="ExternalOutput")
    tabA_out = nc.dram_tensor("tabA", [NPAD, 68], FP, kind="ExternalOutput")
    tabB_out = nc.dram_tensor("tabB", [NPAD, 68], FP, kind="ExternalOutput")

    with tile.TileContext(nc) as tc:
        with tc.tile_pool(name="cst", bufs=1) as cst, \
             tc.tile_pool(name="st", bufs=1) as st, \
             tc.tile_pool(name="g", bufs=2) as gp, \
             tc.tile_pool(name="wk", bufs=2) as wk, \
             tc.tile_pool(name="ps", bufs=4, space="PSUM") as ps, \
             tc.tile_pool(name="psA", bufs=1, space="PSUM") as psA, \
             tc.tile_pool(name="psB", bufs=2, space="PSUM") as psB, \
             tc.tile_pool(name="psG", bufs=1, space="PSUM") as psG, \
             tc.tile_pool(name="dr", bufs=1, space="DRAM") as dr:
            h_t = st.tile([64, NPAD], FP)
            p_t = st.tile([3, NPAD], FP)
            gi_t = cst.tile([128, NCALL], mybir.dt.int32)
            trow_t = cst.tile([1, 512], FP)
            ones_t = cst.tile([1, 128], FP)
            aggvE_d = dr.tile([16, NVV], FP)
            aggvT_d = dr.tile([3, NVV], FP)
            wab0_t = cst.tile([1, 136], FP)
            wabh_t = cst.tile([64, 136], FP)
            wabp_t = cst.tile([3, 136], FP)
            wabq_t = cst.tile([3, 136], FP)
            ew2_t = cst.tile([64, 32], FP)
            ew3_t = cst.tile([32, 16], FP)
            cw1_t = cst.tile([16, 32], FP)
            cw2_t = cst.tile([32, 32], FP)
            cw3_t = cst.tile([32, 3], FP)
            nw1t_t = cst.tile([1, 128], FP)
            nw1h_t = cst.tile([64, 128], FP)
            nw1a_t = cst.tile([16, 128], FP)
            nw2_t = cst.tile([128, 128], FP)
            nw3_t = cst.tile([128, 64], FP)
            bias_t = cst.tile([128, 8], FP)
            wr_t = cst.tile([3, 64], FP)
            e3_t = cst.tile([3, 19], FP)
            ident_t = cst.tile([128, 128], FP)
            aggrE_t = st.tile([16, NPAD], FP)
            aggrT_t = st.tile([3, NPAD], FP)
            for ap, src in [(h_t, h_in), (p_t, p_in), (gi_t, gi_in),
                            (trow_t, trow_in),
                            (wab0_t, wab0_in), (wabh_t, wabh_in),
                            (wabp_t, wabp_in), (wabq_t, wabq_in),
                            (ew2_t, ew2_in), (ew3_t, ew3_in), (cw1_t, cw1_in),
                            (cw2_t, cw2_in), (cw3_t, cw3_in),
                            (nw1t_t, nw1t_in), (nw1h_t, nw1h_in), (nw1a_t, nw1a_in),
                            (nw2_t, nw2_in), (nw3_t, nw3_in), (bias_t, bias_in),
                            (wr_t, wr_in), (e3_t, e3_in)]:
                nc.sync.dma_start(out=ap[:], in_=src[:])
            nc.vector.memset(ones_t[:], 1.0)
            from concourse.masks import make_identity
            make_identity(nc, ident_t[:])

            for w in range(nwin_real):
                avw = wk.tile([128, 4 * 68], FP, tag="wD")
                nc.sync.dma_start(out=avw[:], in_=av_in[:, (4 * w) * 68:(4 * w + 4) * 68])
                awp = psA.tile([68, 512], FP, tag="awp")
                for q in range(4):
                    nc.tensor.transpose(out=awp[:, q * 128:(q + 1) * 128],
                                        in_=avw[:, q * 68:(q + 1) * 68],
                                        identity=ident_t[:])
                aw = wk.tile([68, 512], FP, tag="aw")
                nc.vector.tensor_copy(out=aw[:], in_=awp[:])
                pagg = psG.tile([19, 512], FP, tag="agg")
                for s in range(SLOTS):
                    ch = w * SLOTS + s
                    base = ch * 512
                    zbp = psB.tile([68, 512], FP, tag="zbp")
                    gb = gp.tile([128, 4 * 68], FP, tag="gb")
                    for q in range(4):
                        nc.gpsimd.indirect_dma_start(
                            out=gb[:, q * 68:(q + 1) * 68], out_offset=None, in_=bt_in[:],
                            in_offset=bass.IndirectOffsetOnAxis(
                                ap=gi_t[:, ch * 4 + q:ch * 4 + q + 1], axis=0))
                    for q in range(4):
                        nc.tensor.transpose(out=zbp[:, q * 128:(q + 1) * 128],
                                            in_=gb[:, q * 68:(q + 1) * 68], identity=ident_t[:])
                    epc = wk.tile([64, 512], FP, tag="wG")
                    mkc = wk.tile([16, 512], FP, tag="wM")
                    mkt = wk.tile([3, 512], FP, tag="wM2")
                    nc.sync.dma_start(out=epc[:], in_=ep_in[:, base:base + 512])
                    nc.sync.dma_start(out=mkc[:], in_=maskE_in[:, base:base + 512])
                    nc.sync.dma_start(out=mkt[:], in_=maskT_in[:, base:base + 512])
                    pp = wk.tile([3, 512], FP, tag="tpp")
                    nc.vector.tensor_tensor(out=pp[:], in0=aw[64:67, :], in1=zbp[64:67, :], op=mybir.AluOpType.mult)
                    pcr = ps.tile([128, 512], FP, tag="pe")
                    nc.tensor.matmul(out=pcr[0:64, :], lhsT=wr_t[:], rhs=pp[:],
                                     start=True, stop=True)
                    z = wk.tile([64, 512], FP, tag="tz")
                    nc.vector.tensor_tensor(out=z[:], in0=aw[0:64, :], in1=zbp[0:64, :], op=mybir.AluOpType.add)
                    nc.vector.tensor_tensor(out=z[:], in0=z[:], in1=epc[:], op=mybir.AluOpType.add)
                    nc.vector.tensor_tensor(out=z[:], in0=z[:], in1=pcr[0:64, :], op=mybir.AluOpType.add)
                    h1 = wk.tile([64, 512], FP, tag="th1")
                    _lrelu(nc, h1[:], z[:], wk, (64, 512))
                    cdf = wk.tile([3, 512], FP, tag="tcdf")
                    nc.vector.tensor_tensor(out=cdf[:], in0=aw[64:67, :], in1=zbp[64:67, :], op=mybir.AluOpType.subtract)
                    p2 = ps.tile([128, 512], FP, tag="pe")
                    nc.tensor.matmul(out=p2[0:32, :], lhsT=ew2_t[:], rhs=h1[:], start=True, stop=True)
                    h2 = wk.tile([32, 512], FP, tag="th2")
                    nc.vector.tensor_tensor(out=h2[:], in0=p2[0:32, :], in1=bias_t[0:32, 0:1].to_broadcast([32, 512]), op=mybir.AluOpType.add)
                    _lrelu(nc, h2[:], h2[:], wk, (32, 512))
                    p3 = ps.tile([128, 512], FP, tag="pe")
                    nc.tensor.matmul(out=p3[0:16, :], lhsT=ew3_t[:], rhs=h2[:], start=True, stop=True)
                    ef = wk.tile([16, 512], FP, tag="tef")
                    nc.vector.tensor_tensor(out=ef[:], in0=p3[0:16, :], in1=bias_t[0:16, 1:2].to_broadcast([16, 512]), op=mybir.AluOpType.add)
                    pc1 = ps.tile([128, 512], FP, tag="pe")
                    nc.tensor.matmul(out=pc1[0:32, :], lhsT=cw1_t[:], rhs=ef[:], start=True, stop=True)
                    c1 = wk.tile([32, 512], FP, tag="tc1")
                    nc.vector.tensor_tensor(out=c1[:], in0=pc1[0:32, :], in1=bias_t[0:32, 2:3].to_broadcast([32, 512]), op=mybir.AluOpType.add)
                    _lrelu(nc, c1[:], c1[:], wk, (32, 512))
                    pc2 = ps.tile([128, 512], FP, tag="pe")
                    nc.tensor.matmul(out=pc2[0:32, :], lhsT=cw2_t[:], rhs=c1[:], start=True, stop=True)
                    c2 = wk.tile([32, 512], FP, tag="tc2")
                    nc.vector.tensor_tensor(out=c2[:], in0=pc2[0:32, :], in1=bias_t[0:32, 3:4].to_broadcast([32, 512]), op=mybir.AluOpType.add)
                    _lrelu(nc, c2[:], c2[:], wk, (32, 512))
                    pc3 = ps.tile([128, 512], FP, tag="pe")
                    nc.tensor.matmul(out=pc3[0:3, :], lhsT=cw3_t[:], rhs=c2[:], start=True, stop=True)
                    cm = wk.tile([3, 512], FP, tag="tcm")
                    nc.vector.tensor_tensor(out=cm[:], in0=pc3[0:3, :], in1=bias_t[0:3, 4:5].to_broadcast([3, 512]), op=mybir.AluOpType.add)
                    valsE = wk.tile([16, 512], FP, tag="wV")
                    valsT = wk.tile([3, 512], FP, tag="wV2")
                    nc.vector.tensor_tensor(out=valsE[:], in0=ef[:], in1=mkc[:], op=mybir.AluOpType.mult)
                    nc.vector.tensor_tensor(out=valsT[:], in0=cdf[:], in1=cm[:], op=mybir.AluOpType.mult)
                    nc.vector.tensor_tensor(out=valsT[:], in0=valsT[:], in1=mkt[:], op=mybir.AluOpType.mult)
                    nc.tensor.matmul(out=pagg[:], lhsT=ident_t[0:16, 0:19], rhs=valsE[:],
                                     start=(s == 0), stop=False)
                    nc.tensor.matmul(out=pagg[:], lhsT=e3_t[:], rhs=valsT[:],
                                     start=False, stop=(s == SLOTS - 1))
                agw = wk.tile([19, 512], FP, tag="wG")
                nc.vector.tensor_copy(out=agw[:], in_=pagg[:])
                nc.sync.dma_start(out=aggvE_d[:, w * 512:(w + 1) * 512], in_=agw[0:16, :])
                nc.sync.dma_start(out=aggvT_d[:, w * 512:(w + 1) * 512], in_=agw[16:19, :])

            for (dsrc, dst_t, nr) in [(aggvE_d, aggrE_t, 16), (aggvT_d, aggrT_t, 3)]:
                for (k, rstart, vstart, nk) in classes_u:
                    done = 0
                    while done < nk:
                        nsub = min(nk - done, 2048 // k)
                        cmb = st.tile([16, 2048], FP, tag="cmb")
                        nc.sync.dma_start(out=cmb[0:nr, :nsub * k],
                                          in_=dsrc[:, vstart + done * k: vstart + (done + nsub) * k])
                        s3 = cmb[0:nr, :nsub * k].rearrange("p (n k) -> p n k", k=k)
                        rs0 = rstart + done
                        nc.vector.tensor_copy(out=dst_t[:, rs0:rs0 + nsub], in_=s3[:, :, 0])
                        for kk in range(1, k):
                            nc.vector.tensor_tensor(out=dst_t[:, rs0:rs0 + nsub],
                                                    in0=dst_t[:, rs0:rs0 + nsub],
                                                    in1=s3[:, :, kk], op=mybir.AluOpType.add)
                        done += nsub

            hn_t = h_t
            for b in range(NPAD // 512):
                sl = slice(b * 512, (b + 1) * 512)
                pn1 = ps.tile([128, 512], FP, tag="pe")
                nc.tensor.matmul(out=pn1[:], lhsT=nw1t_t[:], rhs=trow_t[:], start=True, stop=False)
                nc.tensor.matmul(out=pn1[:], lhsT=nw1h_t[:], rhs=h_t[:, sl], start=False, stop=False)
                nc.tensor.matmul(out=pn1[:], lhsT=nw1a_t[:], rhs=aggrE_t[:, sl], start=False, stop=True)
                n1 = wk.tile([128, 512], FP, tag="wA")
                nc.vector.tensor_tensor(out=n1[:], in0=pn1[:], in1=bias_t[:, 5:6].to_broadcast([128, 512]), op=mybir.AluOpType.add)
                _lrelu(nc, n1[:], n1[:], wk, (128, 512))
                pn2 = ps.tile([128, 512], FP, tag="pe")
                nc.tensor.matmul(out=pn2[:], lhsT=nw2_t[:], rhs=n1[:], start=True, stop=True)
                n2 = wk.tile([128, 512], FP, tag="wB")
                nc.vector.tensor_tensor(out=n2[:], in0=pn2[:], in1=bias_t[:, 6:7].to_broadcast([128, 512]), op=mybir.AluOpType.add)
                _lrelu(nc, n2[:], n2[:], wk, (128, 512))
                pn3 = ps.tile([128, 512], FP, tag="pe")
                nc.tensor.matmul(out=pn3[0:64, :], lhsT=nw3_t[:], rhs=n2[:], start=True, stop=True)
                nh = wk.tile([64, 512], FP, tag="wC")
                nc.vector.tensor_tensor(out=nh[:], in0=pn3[0:64, :], in1=bias_t[0:64, 7:8].to_broadcast([64, 512]), op=mybir.AluOpType.add)
                nc.scalar.mul(out=nh[:], in_=nh[:], mul=DT)
                nc.vector.tensor_tensor(out=hn_t[:, sl], in0=h_t[:, sl], in1=nh[:], op=mybir.AluOpType.add)
            nc.scalar.mul(out=aggrT_t[:], in_=aggrT_t[:], mul=DT)
            nc.scalar.mul(out=p_t[:], in_=p_t[:], mul=1.0 + DT)
            nc.vector.tensor_tensor(out=p_t[:], in0=p_t[:], in1=aggrT_t[:], op=mybir.AluOpType.add)
            nc.sync.dma_start(out=h_out[:], in_=hn_t[:])
            nc.sync.dma_start(out=p_out[:], in_=p_t[:])
            # tables via decomposed matmuls: rows [ones, h, p, p^2]
            for b in range(NPAD // 128):
                cs = slice(b * 128, (b + 1) * 128)
                psq = wk.tile([3, 128], FP, tag="wF")
                nc.vector.tensor_tensor(out=psq[:], in0=p_t[:, cs], in1=p_t[:, cs], op=mybir.AluOpType.mult)
                for (rhs_sl, outT, tag) in [(slice(0, 68), tabA_out, "tsA"), (slice(68, 136), tabB_out, "tsB")]:
                    pa = ps.tile([128, 68], FP, tag="pe")
                    nc.tensor.matmul(out=pa[:], lhsT=ones_t[:], rhs=wab0_t[:, rhs_sl], start=True, stop=False)
                    nc.tensor.matmul(out=pa[:], lhsT=hn_t[:, cs], rhs=wabh_t[:, rhs_sl], start=False, stop=False)
                    nc.tensor.matmul(out=pa[:], lhsT=p_t[:, cs], rhs=wabp_t[:, rhs_sl], start=False, stop=False)
                    nc.tensor.matmul(out=pa[:], lhsT=psq[:], rhs=wabq_t[:, rhs_sl], start=False, stop=True)
                    sa = wk.tile([128, 68], FP, tag=tag)
                    nc.vector.tensor_copy(out=sa[:], in_=pa[:])
                    nc.sync.dma_start(out=outT[cs, :], in_=sa[:])
    nc.compile()
    return nc


def kernel(**inputs):
    inputs = {k: np.asarray(v) for k, v in inputs.items()}
    eW1, eb1 = inputs["eW1"].astype(np.float32), inputs["eb1"].astype(np.float32)
    We = eW1[131:135]
    ei = inputs["edge_index"].astype(np.int64)
    row, col = ei[0], ei[1]
    # pass 1: uniform class layout
    per_core = []
    nk_all = {}
    for c in range(NC):
        m = (row // NLOC) == c
        rl = row[m] - c * NLOC
        _, vdeg = _vdeg_of(rl)
        cnts = np.bincount(vdeg[:NLOC])
        per_core.append((m, rl))
        for k in range(1, cnts.size):
            if cnts[k]:
                nk_all[k] = max(nk_all.get(k, 0), int(cnts[k]))
    classes_u = []
    rstart = vstart = 0
    for k in sorted(nk_all):
        classes_u.append((k, rstart, vstart, nk_all[k]))
        rstart += nk_all[k]
        vstart += nk_all[k] * k
    assert rstart <= NPAD, rstart
    assert vstart <= NVV, vstart
    cores = []
    ea = inputs["edge_attr"].astype(np.float32)
    for c in range(NC):
        m, rl = per_core[c]
        cores.append(_prep_core(rl, col[m], np.nonzero(m)[0], ea, We, classes_u))
    # translate gather idx to table rows (class-permuted global)
    invs = [cd["inv_order"] for cd in cores]
    for cd in cores:
        g = cd["gidx"]
        co = g // NLOC
        lo = g % NLOC
        grow = np.zeros(ES, np.int64)
        for c2 in range(NC):
            mm = co == c2
            grow[mm] = c2 * NPAD + invs[c2][lo[mm]]
        cd["grow"] = grow.reshape(ES // 128, 128).T.astype(np.int32).copy()
        ar = np.arange(ES)
        virt_of_pos = (ar // (8 * VW)) * VW + (ar % VW)
        cinv = (1.0 / cd["cnt"]).astype(np.float32)[cd["v2r"][virt_of_pos]]
        m19 = np.zeros((19, ES), np.float32)
        m19[0:16] = cd["mask"]
        m19[16:19] = cd["mask"] * cinv
        cd["mask2"] = m19

    times = np.linspace(0.0, 2.0, 4).astype(np.float32)
    embW = inputs["emb_W"].astype(np.float32)
    embB = inputs["emb_b"].astype(np.float32).reshape(64, 1)
    wabs = [np.concatenate(_build_wab(float(t), eW1, eb1), axis=1) for t in times]
    bias = np.zeros((128, 8), np.float32)
    bias[0:32, 0] = inputs["eb2"]; bias[0:16, 1] = inputs["eb3"]
    bias[0:32, 2] = inputs["cb1"]; bias[0:32, 3] = inputs["cb2"]
    bias[0:3, 4] = inputs["cb3"]; bias[:, 5] = inputs["nb1"]
    bias[:, 6] = inputs["nb2"]; bias[0:64, 7] = inputs["nb3"]
    wr = eW1[130].reshape(64, 1).astype(np.float32)
    ones_row = np.ones((1, NPAD), np.float32)

    global _DBG_CLASSES
    _DBG_CLASSES = classes_u
    p0 = build_p0()
    nv_used = max(vs + k * nk for (k, _, vs, nk) in classes_u)
    nwin_real = (nv_used + VW - 1) // VW
    psp = build_ps(classes_u, nwin_real)

    x = inputs["x"].astype(np.float32); pos = inputs["pos"].astype(np.float32)
    in0 = []
    for c in range(NC):
        od = cores[c]["order"]
        xs = np.zeros((NPAD, F), np.float32); xs[:NLOC] = x[c*NLOC:(c+1)*NLOC]
        pp = np.zeros((NPAD, 3), np.float32); pp[:NLOC] = pos[c*NLOC:(c+1)*NLOC]
        in0.append({"x_fm": np.ascontiguousarray(xs[od].T), "pos_fm": np.ascontiguousarray(pp[od].T),
                    "embW": embW, "embB": embB, "wab0": wabs[0][0:1], "wabh": wabs[0][1:65],
                    "wabp": wabs[0][65:68], "wabq": wabs[0][68:71], "ones": ones_row})
    global LAST_EXEC_NS
    _tot = 0
    r0 = bass_utils.run_bass_kernel_spmd(p0, in0, core_ids=list(range(NC)), trace=TRACE)
    if TRACE and r0.exec_time_ns:
        _tot += r0.exec_time_ns
    h_fm = [r0.results[c]["h0"] for c in range(NC)]
    p_fm = [in0[c]["pos_fm"] for c in range(NC)]
    tabA = [r0.results[c]["tabA"] for c in range(NC)]
    tabB = [r0.results[c]["tabB"] for c in range(NC)]

    out = np.zeros((4, N, H), np.float32)
    for c in range(NC):
        inv = cores[c]["inv_order"]
        out[0, c*NLOC:(c+1)*NLOC] = h_fm[c].T[inv[:NLOC]]

    ew2 = inputs["eW2"].astype(np.float32)
    for step in range(3):
        t = float(times[step])
        btab = np.ascontiguousarray(np.concatenate(tabB, axis=0))
        in_s = []
        for c in range(NC):
            cd = cores[c]
            avirt = tabA[c][cd["v2r"]]                  # [NVV, 68]
            avs = np.ascontiguousarray(
                avirt.reshape(NVV // 128, 128, 68).transpose(1, 0, 2).reshape(128, -1))
            in_s.append({
                "h": h_fm[c], "p": p_fm[c], "avirt": avs, "btab": btab,
                "gidx": cd["grow"],
                "maskE": cd["mask2"][0:16], "maskT": cd["mask2"][16:19],
                "eproj": np.ascontiguousarray(cd["eproj"].T),
                "cnti": (1.0 / cd["cnt"]).astype(np.float32).reshape(1, NPAD),
                "trow": np.full((1, 512), t, np.float32),
                "wab0": wabs[step + 1][0:1], "wabh": wabs[step + 1][1:65],
                "wabp": wabs[step + 1][65:68], "wabq": wabs[step + 1][68:71],
                "ew2": ew2, "ew3": inputs["eW3"].astype(np.float32),
                "cw1": inputs["cW1"].astype(np.float32),
                "cw2": inputs["cW2"].astype(np.float32),
                "cw3": inputs["cW3"].astype(np.float32),
                "nw1t": inputs["nW1"][0:1].astype(np.float32),
                "nw1h": inputs["nW1"][1:65].astype(np.float32),
                "nw1a": inputs["nW1"][65:81].astype(np.float32),
                "nw2": inputs["nW2"].astype(np.float32),
                "nw3": inputs["nW3"].astype(np.float32),
                "biases": bias, "wr3": np.tile(-2.0 * wr.T, (3, 1)).copy(),
                "e3": np.eye(19, dtype=np.float32)[16:19].copy(),
            })
        rs = bass_utils.run_bass_kernel_spmd(psp, in_s, core_ids=list(range(NC)), trace=TRACE)
        if TRACE and rs.exec_time_ns:
            _tot += rs.exec_time_ns
        for c in range(NC):
            h_fm[c] = rs.results[c]["h_new"]
            p_fm[c] = rs.results[c]["p_new"]
            tabA[c] = rs.results[c]["tabA"]
            tabB[c] = rs.results[c]["tabB"]
            inv = cores[c]["inv_order"]
            out[step + 1, c*NLOC:(c+1)*NLOC] = h_fm[c].T[inv[:NLOC]]
    LAST_EXEC_NS = _tot if TRACE else None
    return out


# revision 18
# speedup vs baseline: 1.0365x; 1.0007x over previous
"""EGNN continuous ODE on 8 Trainium2 NeuronCores (Bass/Tile).

Nodes sharded 8-way (per-core class-ordered with padding), edges sharded by
row-core into an 8-slot virtual-node grid. Per Euler step one SPMD launch:
B-side rows fetched with 128-row indirect DMA gathers + PE transposes,
edge MLP feature-major on PE, slot aggregation via PSUM-accumulating matmuls,
virtual->real combine, node MLP, and next-step table build. Host glues the
4 launches (global B-table assembly, virtual expansion, permutations).
"""
import sys
sys.path.insert(0, '/opt/trn_rl_repo')
import numpy as np
from concourse import bass, tile, mybir, bass_utils, bacc

H = 64
F = 128
N = 50000
NC = 8
NLOC = 6250
NPAD = 6656          # padded real nodes per core (13*512)
SLOTS = 8
VW = 512
SLOPE = 0.3
NWIN = 32
NVV = NWIN * VW      # 16384 virtual nodes padded
ES = NVV * SLOTS     # 131072 slot stream
NCALL = ES // 128    # 1024 gather calls / step
FP = mybir.dt.float32
DT = float(2.0 / 3.0)
TRACE = False
LAST_EXEC_NS = None


def _vdeg_of(row_l):
    deg = np.bincount(row_l, minlength=NPAD)
    return deg, np.maximum((deg + SLOTS - 1) // SLOTS, 1)


def _prep_core(row_l, col_g, eids, edge_attr, We, classes_u):
    deg, vdeg = _vdeg_of(row_l)
    # build class-ordered real-node permutation under the UNIFORM layout
    order = np.full(NPAD, -1, np.int64)     # class-pos -> orig local id
    by_k = {}
    for i in np.argsort(vdeg[:NLOC], kind="stable"):
        by_k.setdefault(int(vdeg[i]), []).append(int(i))
    pad_pool = list(range(NLOC, NPAD))      # dummy real nodes (deg 0)
    v2r = np.zeros(NVV, np.int64)           # virtual -> class-pos of real node
    for (k, rstart, vstart, nk) in classes_u:
        mine = by_k.get(k, [])
        assert len(mine) <= nk, (k, len(mine), nk)
        for i, orig in enumerate(mine):
            order[rstart + i] = orig
        for i in range(len(mine), nk):
            order[rstart + i] = pad_pool.pop()
        for i in range(nk * k):
            v2r[vstart + i] = rstart + i // k
    # leftover class positions (beyond classes_u coverage) get remaining pads
    for i in range(NPAD):
        if order[i] < 0:
            order[i] = pad_pool.pop()
    inv_order = np.argsort(order)
    # slot assignment for real edges
    first_v = np.zeros(NPAD, np.int64)      # orig local -> first virtual id
    cls_pos_of = inv_order                  # orig local -> class pos
    # first virtual of each class position
    firstv_of_cpos = np.zeros(NPAD, np.int64)
    for (k, rstart, vstart, nk) in classes_u:
        firstv_of_cpos[rstart:rstart + nk] = vstart + np.arange(nk) * k
    first_v = firstv_of_cpos[cls_pos_of]
    srt = np.argsort(row_l, kind="stable")
    rl_s, cg_s, eid_s = row_l[srt], col_g[srt], eids[srt]
    first_e = np.zeros(NPAD + 1, np.int64)
    np.cumsum(deg, out=first_e[1:])
    within = np.arange(rl_s.size) - first_e[rl_s]
    virt = first_v[rl_s] + within // SLOTS
    slot = within % SLOTS
    pos = (virt // VW) * (SLOTS * VW) + slot * VW + (virt % VW)
    gidx = np.zeros(ES, np.int64)
    mask = np.zeros(ES, np.float32)
    ep = np.zeros((ES, H), np.float32)
    gidx[pos] = cg_s
    mask[pos] = 1.0
    ep[pos] = edge_attr[eid_s] @ We
    cnt = np.maximum(deg, 1).astype(np.float32)[order]   # class order
    return dict(order=order, inv_order=inv_order, v2r=v2r,
                gidx=gidx, mask=mask, eproj=ep, cnt=cnt)


def _build_wab(t, eW1, eb1):
    Wa = np.zeros((71, 68), np.float32)
    Wb = np.zeros((71, 68), np.float32)
    Wr = eW1[130]
    Wa[0, :64] = t * eW1[0] + eb1
    Wa[1:65, :64] = eW1[1:65]
    Wa[65:68, 64:67] = np.eye(3)
    Wa[68:71, :64] = Wr
    Wb[0, :64] = t * eW1[65]
    Wb[1:65, :64] = eW1[66:130]
    Wb[65:68, 64:67] = np.eye(3)
    Wb[68:71, :64] = Wr
    return Wa, Wb


def _lrelu(nc, out_ap, in_ap, pool, shape):
    tmp = pool.tile(list(shape), FP, tag="lrt")
    nc.scalar.mul(out=tmp[:], in_=in_ap, mul=SLOPE)
    nc.vector.tensor_tensor(out=out_ap, in0=in_ap, in1=tmp[:], op=mybir.AluOpType.max)


def _build_ext(nc, pool, h_ap, p_ap, ones_ap):
    ext = pool.tile([71, NPAD], FP, tag="ext")
    nc.vector.tensor_copy(out=ext[0:1, :], in_=ones_ap)
    nc.vector.tensor_copy(out=ext[1:65, :], in_=h_ap)
    nc.vector.tensor_copy(out=ext[65:68, :], in_=p_ap)
    nc.vector.tensor_tensor(out=ext[68:71, :], in0=p_ap, in1=p_ap, op=mybir.AluOpType.mult)
    return ext


def _table_mms(nc, sbuf, psum, ext_t, wab_t, outA, outB):
    for b in range(NPAD // 128):
        pa = psum.tile([128, 68], FP, tag="pe")
        lhsT = ext_t[:, b * 128:(b + 1) * 128]
        nc.tensor.matmul(out=pa[:], lhsT=lhsT, rhs=wab_t[:, 0:68], start=True, stop=True)
        sa = sbuf.tile([128, 68], FP, tag="wE")
        nc.vector.tensor_copy(out=sa[:], in_=pa[:])
        nc.sync.dma_start(out=outA[b * 128:(b + 1) * 128, :], in_=sa[:])
        pb = psum.tile([128, 68], FP, tag="pe")
        nc.tensor.matmul(out=pb[:], lhsT=lhsT, rhs=wab_t[:, 68:136], start=True, stop=True)
        sb2 = sbuf.tile([128, 68], FP, tag="wV")
        nc.vector.tensor_copy(out=sb2[:], in_=pb[:])
        nc.sync.dma_start(out=outB[b * 128:(b + 1) * 128, :], in_=sb2[:])


def build_p0():
    nc = bacc.Bacc("TRN2", target_bir_lowering=False, debug=False, num_devices=NC)
    x_in = nc.dram_tensor("x_fm", [128, NPAD], FP, kind="ExternalInput")
    pos_in = nc.dram_tensor("pos_fm", [3, NPAD], FP, kind="ExternalInput")
    embW_in = nc.dram_tensor("embW", [128, 64], FP, kind="ExternalInput")
    embB_in = nc.dram_tensor("embB", [64, 1], FP, kind="ExternalInput")
    wab0_in = nc.dram_tensor("wab0", [1, 136], FP, kind="ExternalInput")
    wabh_in = nc.dram_tensor("wabh", [64, 136], FP, kind="ExternalInput")
    wabp_in = nc.dram_tensor("wabp", [3, 136], FP, kind="ExternalInput")
    wabq_in = nc.dram_tensor("wabq", [3, 136], FP, kind="ExternalInput")
    ones_in = nc.dram_tensor("ones", [1, NPAD], FP, kind="ExternalInput")
    h0_out = nc.dram_tensor("h0", [64, NPAD], FP, kind="ExternalOutput")
    tabA_out = nc.dram_tensor("tabA", [NPAD, 68], FP, kind="ExternalOutput")
    tabB_out = nc.dram_tensor("tabB", [NPAD, 68], FP, kind="ExternalOutput")
    with tile.TileContext(nc) as tc:
        with tc.tile_pool(name="sb", bufs=1) as sbuf, \
             tc.tile_pool(name="ps", bufs=2, space="PSUM") as psum, \
             tc.tile_pool(name="sb2", bufs=2) as sbuf2:
            x_t = sbuf.tile([128, NPAD], FP)
            pos_t = sbuf.tile([3, NPAD], FP)
            embW_t = sbuf.tile([128, 64], FP)
            embB_t = sbuf.tile([64, 1], FP)
            wab0_t = sbuf.tile([1, 136], FP)
            wabh_t = sbuf.tile([64, 136], FP)
            wabp_t = sbuf.tile([3, 136], FP)
            wabq_t = sbuf.tile([3, 136], FP)
            ones_t = sbuf.tile([1, NPAD], FP)
            h0_t = sbuf.tile([64, NPAD], FP)
            for ap, src in [(x_t, x_in), (pos_t, pos_in), (embW_t, embW_in),
                            (embB_t, embB_in), (wab0_t, wab0_in), (wabh_t, wabh_in),
                            (wabp_t, wabp_in), (wabq_t, wabq_in), (ones_t, ones_in)]:
                nc.sync.dma_start(out=ap[:], in_=src[:])
            for b in range(NPAD // 512):
                pe = psum.tile([64, 512], FP, tag="pe")
                nc.tensor.matmul(out=pe[:], lhsT=embW_t[:],
                                 rhs=x_t[:, b * 512:(b + 1) * 512], start=True, stop=True)
                nc.vector.tensor_tensor(out=h0_t[:, b * 512:(b + 1) * 512],
                                        in0=pe[:], in1=embB_t[:].to_broadcast([64, 512]),
                                        op=mybir.AluOpType.add)
            for b in range(NPAD // 128):
                cs = slice(b * 128, (b + 1) * 128)
                psq = sbuf2.tile([3, 128], FP, tag="psq")
                nc.vector.tensor_tensor(out=psq[:], in0=pos_t[:, cs], in1=pos_t[:, cs], op=mybir.AluOpType.mult)
                for (rhs_sl, outT, tag) in [(slice(0, 68), tabA_out, "tsA"), (slice(68, 136), tabB_out, "tsB")]:
                    pa = psum.tile([128, 68], FP, tag="pe2")
                    nc.tensor.matmul(out=pa[:], lhsT=ones_t[:, cs], rhs=wab0_t[:, rhs_sl], start=True, stop=False)
                    nc.tensor.matmul(out=pa[:], lhsT=h0_t[:, cs], rhs=wabh_t[:, rhs_sl], start=False, stop=False)
                    nc.tensor.matmul(out=pa[:], lhsT=pos_t[:, cs], rhs=wabp_t[:, rhs_sl], start=False, stop=False)
                    nc.tensor.matmul(out=pa[:], lhsT=psq[:], rhs=wabq_t[:, rhs_sl], start=False, stop=True)
                    sa = sbuf2.tile([128, 68], FP, tag=tag)
                    nc.vector.tensor_copy(out=sa[:], in_=pa[:])
                    nc.sync.dma_start(out=outT[cs, :], in_=sa[:])
            nc.sync.dma_start(out=h0_out[:], in_=h0_t[:])
    nc.compile()
    return nc


def build_ps(classes_u, nwin_real):
    nc = bacc.Bacc("TRN2", target_bir_lowering=False, debug=False, num_devices=NC)
    h_in = nc.dram_tensor("h", [64, NPAD], FP, kind="ExternalInput")
    p_in = nc.dram_tensor("p", [3, NPAD], FP, kind="ExternalInput")
    av_in = nc.dram_tensor("avirt", [128, (NVV // 128) * 68], FP, kind="ExternalInput")
    bt_in = nc.dram_tensor("btab", [NC * NPAD, 68], FP, kind="ExternalInput")
    gi_in = nc.dram_tensor("gidx", [128, NCALL], mybir.dt.int32, kind="ExternalInput")
    maskE_in = nc.dram_tensor("maskE", [16, ES], FP, kind="ExternalInput")
    maskT_in = nc.dram_tensor("maskT", [3, ES], FP, kind="ExternalInput")
    ep_in = nc.dram_tensor("eproj", [64, ES], FP, kind="ExternalInput")
    trow_in = nc.dram_tensor("trow", [1, 512], FP, kind="ExternalInput")
    wab0_in = nc.dram_tensor("wab0", [1, 136], FP, kind="ExternalInput")
    wabh_in = nc.dram_tensor("wabh", [64, 136], FP, kind="ExternalInput")
    wabp_in = nc.dram_tensor("wabp", [3, 136], FP, kind="ExternalInput")
    wabq_in = nc.dram_tensor("wabq", [3, 136], FP, kind="ExternalInput")
    ew2_in = nc.dram_tensor("ew2", [64, 32], FP, kind="ExternalInput")
    ew3_in = nc.dram_tensor("ew3", [32, 16], FP, kind="ExternalInput")
    cw1_in = nc.dram_tensor("cw1", [16, 32], FP, kind="ExternalInput")
    cw2_in = nc.dram_tensor("cw2", [32, 32], FP, kind="ExternalInput")
    cw3_in = nc.dram_tensor("cw3", [32, 3], FP, kind="ExternalInput")
    nw1t_in = nc.dram_tensor("nw1t", [1, 128], FP, kind="ExternalInput")
    nw1h_in = nc.dram_tensor("nw1h", [64, 128], FP, kind="ExternalInput")
    nw1a_in = nc.dram_tensor("nw1a", [16, 128], FP, kind="ExternalInput")
    nw2_in = nc.dram_tensor("nw2", [128, 128], FP, kind="ExternalInput")
    nw3_in = nc.dram_tensor("nw3", [128, 64], FP, kind="ExternalInput")
    bias_in = nc.dram_tensor("biases", [128, 8], FP, kind="ExternalInput")
    wr_in = nc.dram_tensor("wr3", [3, 64], FP, kind="ExternalInput")
    e3_in = nc.dram_tensor("e3", [3, 19], FP, kind="ExternalInput")
    h_out = nc.dram_tensor("h_new", [64, NPAD], FP, kind="ExternalOutput")
    p_out = nc.dram_tensor("p_new", [3, NPAD], FP, kind="ExternalOutput")
    tabA_out = nc.dram_tensor("tabA", [NPAD, 68], FP, kind="ExternalOutput")
    tabB_out = nc.dram_tensor("tabB", [NPAD, 68], FP, kind="ExternalOutput")

    with tile.TileContext(nc) as tc:
        with tc.tile_pool(name="cst", bufs=1) as cst, \
             tc.tile_pool(name="st", bufs=1) as st, \
             tc.tile_pool(name="g", bufs=2) as gp, \
             tc.tile_pool(name="wk", bufs=2) as wk, \
             tc.tile_pool(name="ps", bufs=4, space="PSUM") as ps, \
             tc.tile_pool(name="psA", bufs=1, space="PSUM") as psA, \
             tc.tile_pool(name="psB", bufs=2, space="PSUM") as psB, \
             tc.tile_pool(name="psG", bufs=1, space="PSUM") as psG, \
             tc.tile_pool(name="dr", bufs=1, space="DRAM") as dr:
            h_t = st.tile([64, NPAD], FP)
            p_t = st.tile([3, NPAD], FP)
            gi_t = cst.tile([128, NCALL], mybir.dt.int32)
            trow_t = cst.tile([1, 512], FP)
            ones_t = cst.tile([1, 128], FP)
            aggvE_d = dr.tile([16, NVV], FP)
            aggvT_d = dr.tile([3, NVV], FP)
            wab0_t = cst.tile([1, 136], FP)
            wabh_t = cst.tile([64, 136], FP)
            wabp_t = cst.tile([3, 136], FP)
            wabq_t = cst.tile([3, 136], FP)
            ew2_t = cst.tile([64, 32], FP)
            ew3_t = cst.tile([32, 16], FP)
            cw1_t = cst.tile([16, 32], FP)
            cw2_t = cst.tile([32, 32], FP)
            cw3_t = cst.tile([32, 3], FP)
            nw1t_t = cst.tile([1, 128], FP)
            nw1h_t = cst.tile([64, 128], FP)
            nw1a_t = cst.tile([16, 128], FP)
            nw2_t = cst.tile([128, 128], FP)
            nw3_t = cst.tile([128, 64], FP)
            bias_t = cst.tile([128, 8], FP)
            wr_t = cst.tile([3, 64], FP)
            e3_t = cst.tile([3, 19], FP)
            ident_t = cst.tile([128, 128], FP)
            aggrE_t = st.tile([16, NPAD], FP)
            aggrT_t = st.tile([3, NPAD], FP)
            for ap, src in [(h_t, h_in), (p_t, p_in), (gi_t, gi_in),
                            (trow_t, trow_in),
                            (wab0_t, wab0_in), (wabh_t, wabh_in),
                            (wabp_t, wabp_in), (wabq_t, wabq_in),
                            (ew2_t, ew2_in), (ew3_t, ew3_in), (cw1_t, cw1_in),
                            (cw2_t, cw2_in), (cw3_t, cw3_in),
                            (nw1t_t, nw1t_in), (nw1h_t, nw1h_in), (nw1a_t, nw1a_in),
                            (nw2_t, nw2_in), (nw3_t, nw3_in), (bias_t, bias_in),
                            (wr_t, wr_in), (e3_t, e3_in)]:
                nc.sync.dma_start(out=ap[:], in_=src[:])
            nc.vector.memset(ones_t[:], 1.0)
            from concourse.masks import make_identity
            make_identity(nc, ident_t[:])

            for w in range(nwin_real):
                avw = wk.tile([128, 4 * 68], FP, tag="wD")
                nc.sync.dma_start(out=avw[:], in_=av_in[:, (4 * w) * 68:(4 * w + 4) * 68])
                awp = psA.tile([68, 512], FP, tag="awp")
                for q in range(4):
                    nc.tensor.transpose(out=awp[:, q * 128:(q + 1) * 128],
                                        in_=avw[:, q * 68:(q + 1) * 68],
                                        identity=ident_t[:])
                aw = wk.tile([68, 512], FP, tag="aw")
                nc.vector.tensor_copy(out=aw[:], in_=awp[:])
                pagg = psG.tile([19, 512], FP, tag="agg")
                for s in range(SLOTS):
                    ch = w * SLOTS + s
                    base = ch * 512
                    zbp = psB.tile([68, 512], FP, tag="zbp")
                    gb = gp.tile([128, 4 * 68], FP, tag="gb")
                    for q in range(4):
                        nc.gpsimd.indirect_dma_start(
                            out=gb[:, q * 68:(q + 1) * 68], out_offset=None, in_=bt_in[:],
                            in_offset=bass.IndirectOffsetOnAxis(
                                ap=gi_t[:, ch * 4 + q:ch * 4 + q + 1], axis=0))
                    for q in range(4):
                        nc.tensor.transpose(out=zbp[:, q * 128:(q + 1) * 128],
                                            in_=gb[:, q * 68:(q + 1) * 68], identity=ident_t[:])
                    epc = wk.tile([64, 512], FP, tag="wG")
                    mkc = wk.tile([16, 512], FP, tag="wM")
                    mkt = wk.tile([3, 512], FP, tag="wM2")
                    nc.sync.dma_start(out=epc[:], in_=ep_in[:, base:base + 512])
                    nc.sync.dma_start(out=mkc[:], in_=maskE_in[:, base:base + 512])
                    nc.sync.dma_start(out=mkt[:], in_=maskT_in[:, base:base + 512])
                    pp = wk.tile([3, 512], FP, tag="tpp")
                    nc.vector.tensor_tensor(out=pp[:], in0=aw[64:67, :], in1=zbp[64:67, :], op=mybir.AluOpType.mult)
                    pcr = ps.tile([128, 512], FP, tag="pe")
                    nc.tensor.matmul(out=pcr[0:64, :], lhsT=wr_t[:], rhs=pp[:],
                                     start=True, stop=True)
                    z = wk.tile([64, 512], FP, tag="tz")
                    nc.vector.tensor_tensor(out=z[:], in0=aw[0:64, :], in1=zbp[0:64, :], op=mybir.AluOpType.add)
                    nc.vector.tensor_tensor(out=z[:], in0=z[:], in1=epc[:], op=mybir.AluOpType.add)
                    nc.vector.tensor_tensor(out=z[:], in0=z[:], in1=pcr[0:64, :], op=mybir.AluOpType.add)
                    h1 = wk.tile([64, 512], FP, tag="th1")
                    _lrelu(nc, h1[:], z[:], wk, (64, 512))
                    cdf = wk.tile([3, 512], FP, tag="tcdf")
                    nc.vector.tensor_tensor(out=cdf[:], in0=aw[64:67, :], in1=zbp[64:67, :], op=mybir.AluOpType.subtract)
                    p2 = ps.tile([128, 512], FP, tag="pe")
                    nc.tensor.matmul(out=p2[0:32, :], lhsT=ew2_t[:], rhs=h1[:], start=True, stop=True)
                    h2 = wk.tile([32, 512], FP, tag="th2")
                    nc.vector.tensor_tensor(out=h2[:], in0=p2[0:32, :], in1=bias_t[0:32, 0:1].to_broadcast([32, 512]), op=mybir.AluOpType.add)
                    _lrelu(nc, h2[:], h2[:], wk, (32, 512))
                    p3 = ps.tile([128, 512], FP, tag="pe")
                    nc.tensor.matmul(out=p3[0:16, :], lhsT=ew3_t[:], rhs=h2[:], start=True, stop=True)
                    ef = wk.tile([16, 512], FP, tag="tef")
                    nc.vector.tensor_tensor(out=ef[:], in0=p3[0:16, :], in1=bias_t[0:16, 1:2].to_broadcast([16, 512]), op=mybir.AluOpType.add)
                    pc1 = ps.tile([128, 512], FP, tag="pe")
                    nc.tensor.matmul(out=pc1[0:32, :], lhsT=cw1_t[:], rhs=ef[:], start=True, stop=True)
                    c1 = wk.tile([32, 512], FP, tag="tc1")
                    nc.vector.tensor_tensor(out=c1[:], in0=pc1[0:32, :], in1=bias_t[0:32, 2:3].to_broadcast([32, 512]), op=mybir.AluOpType.add)
                    _lrelu(nc, c1[:], c1[:], wk, (32, 512))
                    pc2 = ps.tile([128, 512], FP, tag="pe")
                    nc.tensor.matmul(out=pc2[0:32, :], lhsT=cw2_t[:], rhs=c1[:], start=True, stop=True)
                    c2 = wk.tile([32, 512], FP, tag="tc2")
                    nc.vector.tensor_tensor(out=c2[:], in0=pc2[0:32, :], in1=bias_t[0:32, 3:4].to_broadcast([32, 512]), op=mybir.AluOpType.add)
                    _lrelu(nc, c2[:], c2[:], wk, (32, 512))
                    pc3 = ps.tile([128, 512], FP, tag="pe")
                    nc.tensor.matmul(out=pc3[0:3, :], lhsT=cw3_t[:], rhs=c2[:], start=True, stop=True)
                    cm = wk.tile([3, 512], FP, tag="tcm")
                    nc.vector.tensor_tensor(out=cm[:], in0=pc3[0:3, :], in1=bias_t[0:3, 4:5].to_broadcast([3, 512]), op=mybir.AluOpType.add)
                    valsE = wk.tile([16, 512], FP, tag="wV")
                    valsT = wk.tile([3, 512], FP, tag="wV2")
                    nc.vector.tensor_tensor(out=valsE[:], in0=ef[:], in1=mkc[:], op=mybir.AluOpType.mult)
                    nc.vector.tensor_tensor(out=valsT[:], in0=cdf[:], in1=cm[:], op=mybir.AluOpType.mult)
                    nc.vector.tensor_tensor(out=valsT[:], in0=valsT[:], in1=mkt[:], op=mybir.AluOpType.mult)
                    nc.tensor.matmul(out=pagg[:], lhsT=ident_t[0:16, 0:19], rhs=valsE[:],
                                     start=(s == 0), stop=False)
                    nc.tensor.matmul(out=pagg[:], lhsT=e3_t[:], rhs=valsT[:],
                                     start=False, stop=(s == SLOTS - 1))
                agw = wk.tile([19, 512], FP, tag="wG")
                nc.vector.tensor_copy(out=agw[:], in_=pagg[:])
                nc.sync.dma_start(out=aggvE_d[:, w * 512:(w + 1) * 512], in_=agw[0:16, :])
                nc.sync.dma_start(out=aggvT_d[:, w * 512:(w + 1) * 512], in_=agw[16:19, :])

            for (dsrc, dst_t, nr) in [(aggvE_d, aggrE_t, 16), (aggvT_d, aggrT_t, 3)]:
                for (k, rstart, vstart, nk) in classes_u:
                    done = 0
                    while done < nk:
                        nsub = min(nk - done, 2048 // k)
                        cmb = st.tile([16, 2048], FP, tag="cmb")
                        nc.sync.dma_start(out=cmb[0:nr, :nsub * k],
                                          in_=dsrc[:, vstart + done * k: vstart + (done + nsub) * k])
                        s3 = cmb[0:nr, :nsub * k].rearrange("p (n k) -> p n k", k=k)
                        rs0 = rstart + done
                        nc.vector.tensor_copy(out=dst_t[:, rs0:rs0 + nsub], in_=s3[:, :, 0])
                        for kk in range(1, k):
                            nc.vector.tensor_tensor(out=dst_t[:, rs0:rs0 + nsub],
                                                    in0=dst_t[:, rs0:rs0 + nsub],
                                                    in1=s3[:, :, kk], op=mybir.AluOpType.add)
                        done += nsub

            hn_t = h_t
            for b in range(NPAD // 512):
                sl = slice(b * 512, (b + 1) * 512)
                pn1 = ps.tile([128, 512], FP, tag="pe")
                nc.tensor.matmul(out=pn1[:], lhsT=nw1t_t[:], rhs=trow_t[:], start=True, stop=False)
                nc.tensor.matmul(out=pn1[:], lhsT=nw1h_t[:], rhs=h_t[:, sl], start=False, stop=False)
                nc.tensor.matmul(out=pn1[:], lhsT=nw1a_t[:], rhs=aggrE_t[:, sl], start=False, stop=True)
                n1 = wk.tile([128, 512], FP, tag="wA")
                nc.vector.tensor_tensor(out=n1[:], in0=pn1[:], in1=bias_t[:, 5:6].to_broadcast([128, 512]), op=mybir.AluOpType.add)
                _lrelu(nc, n1[:], n1[:], wk, (128, 512))
                pn2 = ps.tile([128, 512], FP, tag="pe")
                nc.tensor.matmul(out=pn2[:], lhsT=nw2_t[:], rhs=n1[:], start=True, stop=True)
                n2 = wk.tile([128, 512], FP, tag="wB")
                nc.vector.tensor_tensor(out=n2[:], in0=pn2[:], in1=bias_t[:, 6:7].to_broadcast([128, 512]), op=mybir.AluOpType.add)
                _lrelu(nc, n2[:], n2[:], wk, (128, 512))
                pn3 = ps.tile([128, 512], FP, tag="pe")
                nc.tensor.matmul(out=pn3[0:64, :], lhsT=nw3_t[:], rhs=n2[:], start=True, stop=True)
                nh = wk.tile([64, 512], FP, tag="wC")
                nc.vector.tensor_tensor(out=nh[:], in0=pn3[0:64, :], in1=bias_t[0:64, 7:8].to_broadcast([64, 512]), op=mybir.AluOpType.add)
                nc.scalar.mul(out=nh[:], in_=nh[:], mul=DT)
                nc.vector.tensor_tensor(out=hn_t[:, sl], in0=h_t[:, sl], in1=nh[:], op=mybir.AluOpType.add)
            nc.scalar.mul(out=aggrT_t[:], in_=aggrT_t[:], mul=DT)
            nc.scalar.mul(out=p_t[:], in_=p_t[:], mul=1.0 + DT)
            nc.vector.tensor_tensor(out=p_t[:], in0=p_t[:], in1=aggrT_t[:], op=mybir.AluOpType.add)
            nc.sync.dma_start(out=h_out[:], in_=hn_t[:])
            nc.sync.dma_start(out=p_out[:], in_=p_t[:])
            # tables via decomposed matmuls: rows [ones, h, p, p^2]
            for b in range(NPAD // 128):
                cs = slice(b * 128, (b + 1) * 128)
                psq = wk.tile([3, 128], FP, tag="wF")
                nc.vector.tensor_tensor(out=psq[:], in0=p_t[:, cs], in1=p_t[:, cs], op=mybir.AluOpType.mult)
                for (rhs_sl, outT, tag) in [(slice(0, 68), tabA_out, "tsA"), (slice(68, 136), tabB_out, "tsB")]:
                    pa = ps.tile([128, 68], FP, tag="pe")
                    nc.tensor.matmul(out=pa[:], lhsT=ones_t[:], rhs=wab0_t[:, rhs_sl], start=True, stop=False)
                    nc.tensor.matmul(out=pa[:], lhsT=hn_t[:, cs], rhs=wabh_t[:, rhs_sl], start=False, stop=False)
                    nc.tensor.matmul(out=pa[:], lhsT=p_t[:, cs], rhs=wabp_t[:, rhs_sl], start=False, stop=False)
                    nc.tensor.matmul(out=pa[:], lhsT=psq[:], rhs=wabq_t[:, rhs_sl], start=False, stop=True)
                    sa = wk.tile([128, 68], FP, tag=tag)
                    nc.vector.tensor_copy(out=sa[:], in_=pa[:])
                    nc.sync.dma_start(out=outT[cs, :], in_=sa[:])
    nc.compile()
    return nc


def kernel(**inputs):
    inputs = {k: np.asarray(v) for k, v in inputs.items()}
    eW1, eb1 = inputs["eW1"].astype(np.float32), inputs["eb1"].astype(np.float32)
    We = eW1[131:135]
    ei = inputs["edge_index"].astype(np.int64)
    row, col = ei[0], ei[1]
    # pass 1: uniform class layout
    per_core = []
    nk_all = {}
    for c in range(NC):
        m = (row // NLOC) == c
        rl = row[m] - c * NLOC
        _, vdeg = _vdeg_of(rl)
        cnts = np.bincount(vdeg[:NLOC])
        per_core.append((m, rl))
        for k in range(1, cnts.size):
            if cnts[k]:
                nk_all[k] = max(nk_all.get(k, 0), int(cnts[k]))
    classes_u = []
    rstart = vstart = 0
    for k in sorted(nk_all):
        classes_u.append((k, rstart, vstart, nk_all[k]))
        rstart += nk_all[k]
        vstart += nk_all[k] * k
    assert rstart <= NPAD, rstart
    assert vstart <= NVV, vstart
    cores = []
    ea = inputs["edge_attr"].astype(np.float32)
    for c in range(NC):
        m, rl = per_core[c]
        cores.append(_prep_core(rl, col[m], np.nonzero(m)[0], ea, We, classes_u))
    # translate gather idx to table rows (class-permuted global)
    invs = [cd["inv_order"] for cd in cores]
    for cd in cores:
        g = cd["gidx"]
        co = g // NLOC
        lo = g % NLOC
        grow = np.zeros(ES, np.int64)
        for c2 in range(NC):
            mm = co == c2
            grow[mm] = c2 * NPAD + invs[c2][lo[mm]]
        cd["grow"] = grow.reshape(ES // 128, 128).T.astype(np.int32).copy()
        ar = np.arange(ES)
        virt_of_pos = (ar // (8 * VW)) * VW + (ar % VW)
        cinv = (1.0 / cd["cnt"]).astype(np.float32)[cd["v2r"][virt_of_pos]]
        m19 = np.zeros((19, ES), np.float32)
        m19[0:16] = cd["mask"]
        m19[16:19] = cd["mask"] * cinv
        cd["mask2"] = m19

    times = np.linspace(0.0, 2.0, 4).astype(np.float32)
    embW = inputs["emb_W"].astype(np.float32)
    embB = inputs["emb_b"].astype(np.float32).reshape(64, 1)
    wabs = [np.concatenate(_build_wab(float(t), eW1, eb1), axis=1) for t in times]
    bias = np.zeros((128, 8), np.float32)
    bias[0:32, 0] = inputs["eb2"]; bias[0:16, 1] = inputs["eb3"]
    bias[0:32, 2] = inputs["cb1"]; bias[0:32, 3] = inputs["cb2"]
    bias[0:3, 4] = inputs["cb3"]; bias[:, 5] = inputs["nb1"]
    bias[:, 6] = inputs["nb2"]; bias[0:64, 7] = inputs["nb3"]
    wr = eW1[130].reshape(64, 1).astype(np.float32)
    ones_row = np.ones((1, NPAD), np.float32)

    global _DBG_CLASSES
    _DBG_CLASSES = classes_u
    p0 = build_p0()
    nv_used = max(vs + k * nk for (k, _, vs, nk) in classes_u)
    nwin_real = (nv_used + VW - 1) // VW
    psp = build_ps(classes_u, nwin_real)

    x = inputs["x"].astype(np.float32); pos = inputs["pos"].astype(np.float32)
    in0 = []
    for c in range(NC):
        od = cores[c]["order"]
        xs = np.zeros((NPAD, F), np.float32); xs[:NLOC] = x[c*NLOC:(c+1)*NLOC]
        pp = np.zeros((NPAD, 3), np.float32); pp[:NLOC] = pos[c*NLOC:(c+1)*NLOC]
        in0.append({"x_fm": np.ascontiguousarray(xs[od].T), "pos_fm": np.ascontiguousarray(pp[od].T),
                    "embW": embW, "embB": embB, "wab0": wabs[0][0:1], "wabh": wabs[0][1:65],
                    "wabp": wabs[0][65:68], "wabq": wabs[0][68:71], "ones": ones_row})
    global LAST_EXEC_NS
    _tot = 0
    r0 = bass_utils.run_bass_kernel_spmd(p0, in0, core_ids=list(range(NC)), trace=TRACE)
    if TRACE and r0.exec_time_ns:
        _tot += r0.exec_time_ns
    h_fm = [r0.results[c]["h0"] for c in range(NC)]
    p_fm = [in0[c]["pos_fm"] for c in range(NC)]
    tabA = [r0.results[c]["tabA"] for c in range(NC)]
    tabB = [r0.results[c]["tabB"] for c in range(NC)]

    out = np.zeros((4, N, H), np.float32)
    for c in range(NC):
        inv = cores[c]["inv_order"]
        out[0, c*NLOC:(c+1)*NLOC] = h_fm[c].T[inv[:NLOC]]

    ew2 = inputs["eW2"].astype(np.float32)
    for step in range(3):
        t = float(times[step])
        btab = np.ascontiguousarray(np.concatenate(tabB, axis=0))
        in_s = []
        for c in range(NC):
            cd = cores[c]
            avirt = tabA[c][cd["v2r"]]                  # [NVV, 68]
            avs = np.ascontiguousarray(
                avirt.reshape(NVV // 128, 128, 68).transpose(1, 0, 2).reshape(128, -1))
            in_s.append({
                "h": h_fm[c], "p": p_fm[c], "avirt": avs, "btab": btab,
                "gidx": cd["grow"],
                "maskE": cd["mask2"][0:16], "maskT": cd["mask2"][16:19],
                "eproj": np.ascontiguousarray(cd["eproj"].T),
                "cnti": (1.0 / cd["cnt"]).astype(np.float32).reshape(1, NPAD),
                "trow": np.full((1, 512), t, np.float32),
                "wab0": wabs[step + 1][0:1], "wabh": wabs[step + 1][1:65],
                "wabp": wabs[step + 1][65:68], "wabq": wabs[step + 1][68:71],
                "ew2": ew2, "ew3": inputs["eW3"].astype(np.float32),
                "cw1": inputs["cW1"].astype(np.float32),
                "cw2": inputs["cW2"].astype(np.float32),
                "cw3": inputs["cW3"].astype(np.float32),
                "nw1t": inputs["nW1"][0:1].astype(np.float32),
                "nw1h": inputs["nW1"][1:65].astype(np.float32),
                "nw1a": inputs["nW1"][65:81].astype(np.float32),
                "nw2": inputs["nW2"].astype(np.float32),
                "nw3": inputs["nW3"].astype(np.float32),
                "biases": bias, "wr3": np.tile(-2.0 * wr.T, (3, 1)).copy(),
                "e3": np.eye(19, dtype=np.float32)[16:19].copy(),
            })
        rs = bass_utils.run_bass_kernel_spmd(psp, in_s, core_ids=list(range(NC)), trace=TRACE)
        if TRACE and rs.exec_time_ns:
            _tot += rs.exec_time_ns
        for c in range(NC):
            h_fm[c] = rs.results[c]["h_new"]
            p_fm[c] = rs.results[c]["p_new"]
            tabA[c] = rs.results[c]["tabA"]
            tabB[c] = rs.results[c]["tabB"]
            inv = cores[c]["inv_order"]
            out[step + 1, c*NLOC:(c+1)*NLOC] = h_fm[c].T[inv[:NLOC]]
    LAST_EXEC_NS = _tot if TRACE else None
    return out
